# revision 1
# baseline (speedup 1.0000x reference)
"""Causal self-attention (B=2, T=4096, C=768, H=12) on 8 TRN2 NeuronCores.

Sharding: batch x head-group. Core c handles batch b=c//4 and heads
h0..h0+2 where h0 = 3*(c%4). Each core computes qkv projection for its 3
heads, full causal attention, and a partial output projection; the host
sums the 4 partials per batch and adds the projection bias.

On-chip layout is feature-major ("transposed"): qT/kT [D, T] feed the
scores matmul directly, scores^T [k, q] feeds att@v with v in natural
layout, and the attention output stays transposed to feed the output
projection as the stationary operand (producing natural-layout y).
Matmuls run in float32r (~tf32). The softmax denominator comes free as a
65th "ones" column of v; normalization uses reciprocal_approx_fast + a
gpsimd partition broadcast.
"""

import sys

for _p in ("/opt/trn_rl_repo",):
    if _p not in sys.path:
        sys.path.insert(0, _p)

from contextlib import ExitStack

import numpy as np

import concourse.bass as bass  # noqa: F401  (engine classes referenced via nc)
import concourse.mybir as mybir
import concourse.tile as tile
from concourse import bacc
from concourse.bass_utils import run_bass_kernel_spmd
from concourse.masks import make_identity
from concourse.tile_rust import add_dep_helper

f32 = mybir.dt.float32
f32r = mybir.dt.float32r
AF = mybir.ActivationFunctionType

C = 768
D = 64
N_HEAD = 12
HPC = 3  # heads per core
N_CORES = 8

# wq column slots: q01 | k01 | v01 | (q2 stacked over k2) | v2
SLOTS = [(0, 128), (128, 256), (256, 384), (384, 512), (512, 576)]


def build_nc(T):
    NT = T // 512  # q tiles
    KT = T // 128  # k tiles
    CK = C // 128  # contraction chunks for qkv

    nc = bacc.Bacc("TRN2", target_bir_lowering=False, debug=False,
                   num_devices=N_CORES)
    xt_d = nc.dram_tensor("xt", [C, T], f32r, kind="ExternalInput").ap()
    wq_d = nc.dram_tensor("wq", [C, 576], f32r, kind="ExternalInput").ap()
    bq_d = nc.dram_tensor("bq", [128, 5], f32, kind="ExternalInput").ap()
    wp_d = nc.dram_tensor("wp", [HPC * D, C], f32r, kind="ExternalInput").ap()
    y_d = nc.dram_tensor("y", [T, C], f32, kind="ExternalOutput").ap()
    import os
    dbg = os.environ.get("KDBG") == "1"
    kphase = int(os.environ.get("KPHASE", "4"))
    # internal DRAM scratch for the softmax-reciprocal row broadcast
    rsc_d = nc.dram_tensor("rscratch", [NT * HPC, 512], f32,
                           **({"kind": "ExternalOutput"} if dbg else {})).ap()
    dbg_out = {}
    if dbg:
        for nm, shp in [("d_qAB", [128, T]), ("d_kAB", [128, T]),
                        ("d_qC", [128, T]), ("d_kC", [128, T]),
                        ("d_vaug", [128, KT * 195]),
                        ("d_ao0", [64, T]), ("d_ao1", [64, T]),
                        ("d_ao2", [64, T]), ("d_bc", [64, 512]),
                        ("d_eb", [128, 3072]), ("d_attv", [65, 512])]:
            dbg_out[nm] = nc.dram_tensor(nm, shp, f32, kind="ExternalOutput").ap()

    with tile.TileContext(nc) as tc, ExitStack() as ctx:
        sb = ctx.enter_context(tc.tile_pool(name="sb", bufs=1))

        # persistent tensors (live for the whole kernel)
        bq_sb = sb.tile([128, 5], f32, tag="bq")
        qT_AB = sb.tile([128, T], f32r, tag="qAB")
        kT_AB = sb.tile([128, T], f32r, tag="kAB")
        qT_C = sb.tile([128, T], f32r, tag="qC")
        kT_C = sb.tile([128, T], f32r, tag="kC")
        ident = sb.tile([128, 128], f32, tag="ident")
        ones_f = sb.tile([128, 1], f32, tag="ones")

        nc.sync.dma_start(bq_sb[:], bq_d)
        make_identity(nc, ident[:])
        nc.vector.memset(ones_f[:], 1.0)
        # causal masks for the 4 diagonal-band positions: keep col-p >= 128*r
        cmask = sb.tile([128, 4 * 512], f32, tag="cmask")
        nc.gpsimd.memset(cmask[:], 1.0)
        for r in range(4):
            nc.gpsimd.affine_select(
                cmask[:, r * 512:(r + 1) * 512], cmask[:, r * 512:(r + 1) * 512],
                pattern=[[1, 512]], compare_op=mybir.AluOpType.is_ge, fill=0.0,
                base=-128 * r, channel_multiplier=-1)

        # vaug lives phases 2-3; vpool (inside it) only phases 1-2
        vaugp = ctx.enter_context(tc.tile_pool(name="vaugp", bufs=1))
        es_v = ExitStack()
        vp = es_v.enter_context(tc.tile_pool(name="vpool", bufs=1))
        vT01 = vp.tile([128, T], f32, tag="v01")
        vT2 = vp.tile([64, T], f32, tag="v2")

        # ---------------- phase 1: qkv projection (transposed) --------------
        with tc.tile_pool(name="wqp", bufs=1) as wqp, \
             tc.tile_pool(name="xtp", bufs=12) as xt_pool, \
             tc.tile_pool(name="qkvpsA", bufs=2, space="PSUM") as qkv_psA, \
             tc.tile_pool(name="qkvps", bufs=1, space="PSUM") as qkv_ps:
            wq_sb = [wqp.tile([128, 576], f32r, tag=f"wq{c}", name=f"wq{c}")
                     for c in range(CK)]
            for c in range(CK):
                nc.sync.dma_start(wq_sb[c][:], wq_d[c * 128:(c + 1) * 128, :])
            for j in range(NT):
                jsl = bass.ts(j, 512)
                ps = [qkv_psA.tile([128, 512], f32, tag=f"s{k}", name=f"ps{k}")
                      for k in range(3)]
                ps.append(qkv_ps.tile([128, 512], f32, tag="s3", name="ps3"))
                ps.append(qkv_ps.tile([64, 512], f32, tag="s4", name="ps4"))
                for c in range(CK):
                    xt_t = xt_pool.tile([128, 512], f32r, tag="xt")
                    nc.sync.dma_start(
                        xt_t[:], xt_d[c * 128:(c + 1) * 128, j * 512:(j + 1) * 512])
                    for s, (c0, c1) in enumerate(SLOTS):
                        nc.tensor.matmul(ps[s][:], wq_sb[c][:, c0:c1], xt_t[:],
                                         start=(c == 0), stop=(c == CK - 1))
                nc.vector.tensor_scalar_add(qT_AB[:, jsl], ps[0][:], bq_sb[:, 0:1])
                nc.vector.tensor_scalar_add(kT_AB[:, jsl], ps[1][:], bq_sb[:, 1:2])
                nc.vector.tensor_scalar_add(vT01[:, jsl], ps[2][:], bq_sb[:, 2:3])
                nc.vector.tensor_scalar_add(qT_C[0:64, jsl], ps[3][0:64, :],
                                            bq_sb[0:64, 3:4])
                nc.vector.tensor_scalar_add(kT_C[64:128, jsl], ps[3][64:128, :],
                                            bq_sb[64:128, 3:4])
                nc.vector.tensor_scalar_add(vT2[:, jsl], ps[4][:], bq_sb[0:64, 4:5])
            # duplicate head-2 q/k into the other 64-partition strip
            nc.sync.dma_start(qT_C[64:128, :], qT_C[0:64, :])
            nc.sync.dma_start(kT_C[0:64, :], kT_C[64:128, :])
            if dbg:
                nc.sync.dma_start(dbg_out["d_qAB"], qT_AB[:].bitcast(f32))
                nc.sync.dma_start(dbg_out["d_kAB"], kT_AB[:].bitcast(f32))
                nc.sync.dma_start(dbg_out["d_qC"], qT_C[:].bitcast(f32))
                nc.sync.dma_start(dbg_out["d_kC"], kT_C[:].bitcast(f32))

        # ---------------- phase 2: v -> natural layout + ones column --------
        if kphase >= 2:
          v_aug = vaugp.tile([128, KT * 195], f32r, tag="vaug")
          with tc.tile_pool(name="tps", bufs=3, space="PSUM") as tp_ps:
            for ki in range(KT):
                ksl = bass.ts(ki, 128)
                base = ki * 195
                p01 = tp_ps.tile([128, 128], f32, tag="tp01")
                nc.tensor.transpose(p01[:], vT01[:, ksl], ident[:])
                p2t = tp_ps.tile([128, 64], f32, tag="tp2")
                nc.tensor.transpose(p2t[:], vT2[:, ksl], ident[0:64, 0:64])
                nc.vector.tensor_copy(v_aug[:, base:base + 64], p01[:, 0:64])
                nc.vector.tensor_copy(v_aug[:, base + 65:base + 129], p01[:, 64:128])
                nc.vector.tensor_copy(v_aug[:, base + 130:base + 194], p2t[:])
            ones_cols = v_aug[:].rearrange("p (k c) -> p k c", c=65)[:, :, 64:65]
            nc.vector.tensor_copy(
                ones_cols, ones_f[:, 0:1, None].broadcast_to([128, 3 * KT, 1]))
          if dbg:
              nc.sync.dma_start(dbg_out["d_vaug"], v_aug[:].bitcast(f32))
          es_v.close()  # vT buffers no longer needed

          # ---------------- phase 3: attention -------------------------------
          aop = ctx.enter_context(tc.tile_pool(name="aop", bufs=1))
          aoT = [aop.tile([64, T], f32r, tag=f"aoT{h}", name=f"aoT{h}")
                 for h in range(HPC)]
          with tc.tile_pool(name="scps", bufs=2, space="PSUM") as sc_ps, \
             tc.tile_pool(name="avps", bufs=3, space="PSUM") as av_ps, \
             tc.tile_pool(name="pps", bufs=1, space="PSUM") as pr_ps, \
             tc.tile_pool(name="ebp", bufs=6) as eb_pool, \
             tc.tile_pool(name="wpp", bufs=1) as wpp, \
             tc.tile_pool(name="yp", bufs=3) as y_pool, \
             tc.tile_pool(name="nrm", bufs=3) as nrm:
            wp_sb = [wpp.tile([64, C], f32r, tag=f"wp{h}", name=f"wp{h}")
                     for h in range(HPC)]
            for h in range(HPC):
                nc.sync.dma_start(wp_sb[h][:], wp_d[h * 64:(h + 1) * 64, :])

            def emit_proj(m):
                msl = bass.ts(m, 128)
                y_sb = y_pool.tile([128, C], f32, tag="y", name="ysb")
                for ns in range(2):
                    py = pr_ps.tile([128, 384], f32, tag="py", name="py")
                    for h in range(HPC):
                        nc.tensor.matmul(py[:], aoT[h][:, msl],
                                         wp_sb[h][:, ns * 384:(ns + 1) * 384],
                                         start=(h == 0), stop=(h == HPC - 1))
                    nc.vector.tensor_copy(y_sb[:, ns * 384:(ns + 1) * 384],
                                          py[:])
                nc.sync.dma_start(y_d[m * 128:(m + 1) * 128, :], y_sb[:])

            for j in range(NT if kphase >= 3 else 0):
                jsl = bass.ts(j, 512)
                nk = 4 * j + 4
                for slot in ("AB", "C"):
                    if slot == "AB":
                        heads = [0, 1]
                        group = 1  # k-tiles per round (2 banks each)
                    else:
                        heads = [2]
                        group = 2
                    att = {h: av_ps.tile([65, 512], f32, tag="attv", name=f"attv{h}")
                           for h in heads}
                    for g0 in range(0, nk, group):
                        ks = list(range(g0, min(g0 + group, nk)))
                        nbank = len(ks) * len(heads)
                        pr = sc_ps.tile([128, 1024], f32, tag="sc")
                        banks = []  # (bank, ki, head)
                        for idx, ki in enumerate(ks):
                            ksl = bass.ts(ki, 128)
                            if slot == "AB":
                                for hh in (0, 1):
                                    b = idx * 2 + hh
                                    r0, r1 = 64 * hh, 64 * hh + 64
                                    nc.tensor.matmul(
                                        pr[:, bass.ts(b, 512)],
                                        kT_AB[r0:r1, ksl], qT_AB[r0:r1, jsl],
                                        start=True, stop=True)
                                    banks.append((b, ki, hh))
                            else:
                                strip = idx % 2
                                r0, r1 = 64 * strip, 64 * strip + 64
                                nc.tensor.matmul(
                                    pr[:, bass.ts(idx, 512)],
                                    kT_C[r0:r1, ksl], qT_C[r0:r1, jsl],
                                    start=True, stop=True)
                                banks.append((idx, ki, 2))
                        eb = eb_pool.tile([128, 1024], f32r, tag="eb")
                        nc.scalar.activation(eb[:, 0:nbank * 512],
                                             pr[:, 0:nbank * 512],
                                             AF.Exp, scale=0.125)
                        if dbg and j == NT - 1 and slot == "C" and g0 == 0:
                            nc.sync.dma_start(dbg_out["d_eb"][:, 0:nbank * 512],
                                              eb[:, 0:nbank * 512].bitcast(f32))
                        for b, ki, h in banks:
                            if ki >= 4 * j:  # diagonal band: causal mask
                                bsl = bass.ts(b, 512)
                                r = ki - 4 * j
                                nc.vector.tensor_mul(
                                    eb[:, bsl], eb[:, bsl],
                                    cmask[:, bass.ts(r, 512)])
                        for b, ki, h in banks:
                            nc.tensor.matmul(
                                att[h][:], v_aug[:, ki * 195 + 65 * h:
                                                 ki * 195 + 65 * h + 65],
                                eb[:, bass.ts(b, 512)],
                                start=(ki == 0), stop=(ki == nk - 1),
                                skip_group_check=True)
                    for h in heads:
                        if dbg and j == NT - 1 and h == 2:
                            datt = nrm.tile([65, 512], f32, tag="datt")
                            nc.vector.tensor_copy(datt[:], att[h][:])
                            nc.sync.dma_start(dbg_out["d_attv"], datt[:])
                        # denominator row (psum p64) -> sbuf, then broadcast
                        # across 64 partitions via a DRAM round-trip (stride-0
                        # leading dim is DRAM-only). Tile does not dep-track
                        # DRAM, so wire the RAW edge explicitly. The recip runs
                        # after the broadcast: custom-dve ops misbehave at
                        # nonzero base partitions.
                        scrA = nrm.tile([65, 512], f32, tag="scrA")
                        nc.vector.tensor_copy(scrA[64:65, :], att[h][64:65, :])
                        row_d = rsc_d[j * HPC + h, :]
                        wr = nc.sync.dma_start(row_d[None, :], scrA[64:65, :])
                        bc = nrm.tile([64, 512], f32, tag="bc")
                        rd = nc.gpsimd.dma_start(
                            out=bc[:], in_=bass.AP(row_d.tensor, row_d.offset,
                                                   [[0, 64], [1, 512]]))
                        add_dep_helper(rd.ins, wr.ins,
                                       reason="rscratch RAW (dram roundtrip)")
                        rcp = nrm.tile([64, 512], f32, tag="rcp")
                        nc.vector.reciprocal_approx_fast(out=rcp[:], in_=bc[:])
                        nc.vector.tensor_mul(aoT[h][:, jsl], att[h][0:64, :], rcp[:])
                        if dbg and j == NT - 1 and h == 2:
                            nc.sync.dma_start(dbg_out["d_bc"], bc[:])
                for m in range(4 * j, 4 * j + 4):
                    if kphase >= 4:
                        emit_proj(m)

        if dbg:
            for h in range(HPC):
                nc.sync.dma_start(dbg_out[f"d_ao{h}"], aoT[h][:].bitcast(f32))

    nc.compile()
    return nc


_NC_CACHE = {}


def _get_nc(T):
    if T not in _NC_CACHE:
        _NC_CACHE[T] = build_nc(T)
    return _NC_CACHE[T]


def make_core_inputs(x, W_attn, b_attn, W_proj):
    """Host-side prep: per-core input dicts (see module docstring)."""
    B, T, _ = x.shape
    xts = [np.ascontiguousarray(x[b].T) for b in range(B)]
    in_maps = []
    for core in range(N_CORES):
        b = core // (N_CORES // B)
        h0 = HPC * (core % (N_CORES // B))
        ccols = slice(h0 * D, (h0 + 2) * D)      # first two heads
        c2 = slice((h0 + 2) * D, (h0 + 3) * D)   # third head
        # reference splits qkv as (k, q, v): k cols 0:C, q cols C:2C, v 2C:3C
        q01 = W_attn[:, C:2 * C][:, ccols]
        k01 = W_attn[:, 0:C][:, ccols]
        v01 = W_attn[:, 2 * C:3 * C][:, ccols]
        q2 = W_attn[:, C:2 * C][:, c2]
        k2 = W_attn[:, 0:C][:, c2]
        v2 = W_attn[:, 2 * C:3 * C][:, c2]
        wq = np.ascontiguousarray(
            np.concatenate([q01, k01, v01, q2, k2, v2], axis=1))
        bq = np.zeros((128, 5), np.float32)
        bq[:, 0] = b_attn[C:2 * C][ccols]
        bq[:, 1] = b_attn[0:C][ccols]
        bq[:, 2] = b_attn[2 * C:3 * C][ccols]
        bq[0:64, 3] = b_attn[C:2 * C][c2]
        bq[64:128, 3] = b_attn[0:C][c2]
        bq[0:64, 4] = b_attn[2 * C:3 * C][c2]
        wp = np.ascontiguousarray(W_proj[h0 * D:(h0 + HPC) * D, :])
        in_maps.append({"xt": xts[b], "wq": wq, "bq": bq, "wp": wp})
    return in_maps


def kernel(x, W_attn, b_attn, W_proj, b_proj):
    x = np.asarray(x, dtype=np.float32)
    W_attn = np.asarray(W_attn, dtype=np.float32)
    b_attn = np.asarray(b_attn, dtype=np.float32)
    W_proj = np.asarray(W_proj, dtype=np.float32)
    b_proj = np.asarray(b_proj, dtype=np.float32)
    B, T, _ = x.shape

    nc = _get_nc(T)
    in_maps = make_core_inputs(x, W_attn, b_attn, W_proj)
    res = None
    for attempt in range(3):
        try:
            res = run_bass_kernel_spmd(nc, in_maps, list(range(N_CORES)))
            break
        except Exception:
            # transient NRT_EXEC_UNIT_UNRECOVERABLE has been observed once
            # after a prior crashed process; a retry succeeds
            if attempt == 2:
                raise
    global LAST_RUN
    LAST_RUN = res

    gpb = N_CORES // B
    out = np.empty((B, T, C), np.float32)
    for b in range(B):
        acc = res.results[b * gpb]["y"].astype(np.float32)
        for g in range(1, gpb):
            acc = acc + res.results[b * gpb + g]["y"]
        out[b] = acc + b_proj[None, :]
    return out



# revision 34
# speedup vs baseline: 1.3471x; 1.3471x over previous
"""Causal self-attention (B=2, T=4096, C=768, H=12) on 8 TRN2 NeuronCores.

Sharding: batch x head-group. Core c handles batch b=c//4 and heads
h0..h0+2 where h0 = 3*(c%4). Each core computes the qkv projection for
its 3 heads, full causal attention, and a partial output projection; the
host sums the 4 partials per batch and adds the projection bias.

v2 design (all matmul inputs bf16):
- q/k are produced transposed ([d, T]); v is produced directly in
  natural layout [k, d] per 128-k-tile (lhsT = x^T tile, rhs = Wv
  chunk), with the v bias folded in via a ones-row matmul. No v
  transposes.
- Scores keep the [k-part, q-free] orientation; exp runs on the scalar
  engine over groups of up to 3 PSUM banks into SBUF bf16 tiles (eb).
  Above-diagonal q columns are trimmed from the score matmuls (exp of
  the resulting zeroed columns is never consumed).
- att@v is reoriented to out [q-part, 65]: lhsT = eb column block,
  rhs = v_aug [k, 65] (v plus a ones column). The 65th output column
  is the softmax denominator, per-partition, so normalization is a
  reciprocal + tensor_scalar multiply - no partition broadcast.
- The normalized attention output [q, d] is transposed back (PE, bf16)
  and packed into aoT01 [128, T] / aoT2 [64, T] so the output
  projection contracts 128+64 at a time.
- PSUM: one manually-rotated 6-bank "ring" tile serves scores, qkv
  slots, transposes and proj outputs (each slot is a full bank, so
  matmul start=True bank-zeroing never clobbers live data); 2 banks
  hold att accumulators.
- Scheduling: exp on Act (~212us) and matmul rows on PE (~200us) are
  the co-bottlenecks. All non-score work is queued as (cost, closure)
  items and interleaved between score groups against a per-round time
  budget, with scores emitted one group ahead of exp (including across
  stream boundaries) so the in-order PE queue always has the next
  score group before a deferred batch, and the Act engine never
  starves. Pool does the qkv PSUM->SBUF copies (+bias), causal masks,
  normalize multiplies and y copies; DVE does reciprocals and aoT
  copies.
"""

import sys

for _p in ("/opt/trn_rl_repo",):
    if _p not in sys.path:
        sys.path.insert(0, _p)

from collections import deque
from contextlib import ExitStack

import numpy as np

import concourse.bass as bass  # noqa: F401
import concourse.mybir as mybir
import concourse.tile as tile
from concourse import bacc
from concourse.bass_utils import run_bass_kernel_spmd
from concourse.masks import make_identity

f32 = mybir.dt.float32
bf16 = mybir.dt.bfloat16
AF = mybir.ActivationFunctionType

C = 768
D = 64
N_HEAD = 12
HPC = 3  # heads per core
N_CORES = 8


def build_nc(T):
    NT = T // 512  # q tiles
    KT = T // 128  # k tiles
    CK = C // 128  # contraction chunks

    nc = bacc.Bacc("TRN2", target_bir_lowering=False, debug=False,
                   num_devices=N_CORES)
    xt_d = nc.dram_tensor("xt", [C, T], bf16, kind="ExternalInput").ap()
    wqv_d = nc.dram_tensor("wqv", [C, 576], bf16, kind="ExternalInput").ap()
    bv_d = nc.dram_tensor("bv", [1, 192], bf16, kind="ExternalInput").ap()
    bqk_d = nc.dram_tensor("bqk", [128, 3], f32, kind="ExternalInput").ap()
    wp01_d = nc.dram_tensor("wp01", [128, C], bf16, kind="ExternalInput").ap()
    wp2_d = nc.dram_tensor("wp2", [64, C], bf16, kind="ExternalInput").ap()
    y_d = nc.dram_tensor("y", [T, C], f32, kind="ExternalOutput").ap()

    with tile.TileContext(nc) as tc, ExitStack() as ctx:
        sb = ctx.enter_context(tc.tile_pool(name="sb", bufs=1))

        # ---- persistent SBUF ----
        wqv_sb = [sb.tile([128, 576], bf16, tag=f"wqv{c}", name=f"wqv{c}")
                  for c in range(CK)]
        bv_sb = sb.tile([1, 192], bf16, tag="bv")
        bqk_sb = sb.tile([128, 3], f32, tag="bqk")
        wp01_sb = sb.tile([128, C], bf16, tag="wp01")
        wp2_sb = sb.tile([64, C], bf16, tag="wp2")
        ident = sb.tile([128, 128], bf16, tag="ident")
        ones1 = sb.tile([1, 128], bf16, tag="ones1")
        qT_AB = sb.tile([128, T], bf16, tag="qAB")
        kT_AB = sb.tile([128, T], bf16, tag="kAB")
        qT_C = sb.tile([64, T], bf16, tag="qC")
        kT_C = sb.tile([128, T], bf16, tag="kC")
        v_aug = sb.tile([128, KT * 195], bf16, tag="vaug")
        aoT_pool = ctx.enter_context(tc.tile_pool(name="aotp", bufs=3))

        # ---- pools ----
        xt_pool = ctx.enter_context(tc.tile_pool(name="xtp", bufs=13))
        y_pool = ctx.enter_context(tc.tile_pool(name="yp", bufs=6))
        eb_pool = ctx.enter_context(tc.tile_pool(name="ebp", bufs=3))
        attn_pool = ctx.enter_context(tc.tile_pool(name="attnp", bufs=3))
        rcp_pool = ctx.enter_context(tc.tile_pool(name="rcpp", bufs=4))
        ring_pool = ctx.enter_context(
            tc.tile_pool(name="ringp", bufs=3, space="PSUM"))
        att_pool = ctx.enter_context(
            tc.tile_pool(name="attp", bufs=2, space="PSUM"))

        # startup: x tile DMAs for j=0 first, then weights in use order
        xt0 = [xt_pool.tile([128, 512], bf16, tag="xt", name="xt0")
               for _ in range(CK)]
        for c in range(CK):
            nc.sync.dma_start(xt0[c][:], xt_d[c * 128:(c + 1) * 128, 0:512])
            nc.scalar.dma_start(out=wqv_sb[c][:],
                                in_=wqv_d[c * 128:(c + 1) * 128, :])
        nc.sync.dma_start(bqk_sb[:], bqk_d)
        nc.sync.dma_start(bv_sb[:], bv_d)
        nc.sync.dma_start(wp01_sb[:], wp01_d)
        nc.sync.dma_start(wp2_sb[:], wp2_d)
        make_identity(nc, ident[:])
        nc.vector.memset(ones1[:], 1.0)
        va_r = v_aug[:].rearrange("p (k c) -> p k c", c=195)
        for h in range(HPC):
            nc.gpsimd.memset(va_r[:, :, 65 * h + 64:65 * h + 65], 1.0)

        def next_slot():
            """Claim a 2-bank PSUM slot; pool rotation provides exact
            per-slot WAR/RAW dependencies."""
            return ring_pool.tile([128, 1024], f32, tag="ring", name="slot")

        # ------------- deferred-work queue -------------
        # items are (pe_cost_ns, closure); popped between score groups
        # against a per-round budget so the next score group is never far
        # behind in the in-order PE queue.
        import os
        sched_dbg = os.environ.get("KDBG_SCHED") == "1"
        dq = deque()
        dq_cost = [0.0]
        gen_state = {"cur": 0}

        def push(cost, fn):
            dq.append((cost, fn, gen_state["cur"]))
            dq_cost[0] += cost

        def pop_budget(budget):
            spent = 0.0
            while dq and spent < budget:
                cost, fn, g = dq.popleft()
                dq_cost[0] -= cost
                fn()
                spent += cost

        # ------------- emission helpers -------------
        def emit_qkv(j, xt_pre=None):
            """Queue q/k/v production for q-tile j."""
            jsl = bass.ts(j, 512)
            xt_t = list(xt_pre) if xt_pre is not None else [None] * CK

            def clo_dma():
                for c in range(CK):
                    xt_t[c] = xt_pool.tile([128, 512], bf16, tag="xt",
                                           name="xt")
                    nc.sync.dma_start(
                        xt_t[c][:],
                        xt_d[c * 128:(c + 1) * 128, j * 512:(j + 1) * 512])

            def mk_qk(s):
                state = {}

                def clo_a():
                    state["sl"] = next_slot()
                    out = state["sl"][:, 0:512]
                    for c in range(3):
                        nc.tensor.matmul(out,
                                         wqv_sb[c][:, s * 128:(s + 1) * 128],
                                         xt_t[c][:], start=(c == 0),
                                         stop=False)

                def clo_b():
                    out = state["sl"][:, 0:512]
                    for c in range(3, CK):
                        nc.tensor.matmul(out,
                                         wqv_sb[c][:, s * 128:(s + 1) * 128],
                                         xt_t[c][:], start=False,
                                         stop=(c == CK - 1))
                    if s == 0:
                        nc.vector.tensor_scalar_add(qT_AB[:, jsl], out,
                                                    bqk_sb[:, 0:1])
                    elif s == 1:
                        nc.vector.tensor_scalar_add(kT_AB[:, jsl], out,
                                                    bqk_sb[:, 1:2])
                    else:
                        nc.vector.tensor_scalar_add(qT_C[:, jsl],
                                                    out[0:64, :],
                                                    bqk_sb[0:64, 2:3])
                        nc.vector.tensor_scalar_add(kT_C[64:128, jsl],
                                                    out[64:128, :],
                                                    bqk_sb[64:128, 2:3])
                        # k2 must also live on partitions 0:64 (score lhsT)
                        nc.sync.dma_start(kT_C[0:64, jsl], kT_C[64:128, jsl])
                return clo_a, clo_b

            def mk_v(kt):
                def clo():
                    ki = 4 * j + kt
                    out = next_slot()[:, 0:192]
                    for c in range(CK):
                        nc.tensor.matmul(
                            out, xt_t[c][:, kt * 128:(kt + 1) * 128],
                            wqv_sb[c][:, 384:576], start=(c == 0), stop=False)
                    nc.tensor.matmul(out, ones1[:], bv_sb[:],
                                     start=False, stop=True)
                    dst = va_r[:, ki:ki + 1, :].rearrange(
                        "p k (h c) -> p (k h) c", c=65)[:, :, 0:64]
                    src = out.rearrange("p (h c) -> p h c", c=64)
                    nc.vector.tensor_copy(dst, src)
                return clo

            qk_part = []
            for s in range(3):
                ca, cb = mk_qk(s)
                qk_part.append((640, ca))
                qk_part.append((640, cb))
            v_part = [(600, mk_v(kt)) for kt in range(4)]
            dma_part = [] if xt_pre is not None else [(50, clo_dma)]
            return dma_part, qk_part, v_part

        def head_qk(h):
            if h == 0:
                return kT_AB[0:64, :], qT_AB[0:64, :]
            if h == 1:
                return kT_AB[64:128, :], qT_AB[64:128, :]
            return kT_C[0:64, :], qT_C[:]

        def emit_scores(j, h, grp):
            kt_src, qt_src = head_qk(h)
            slot = next_slot()
            for idx, ki in enumerate(grp):
                r = ki - 4 * j
                t = 128 * r if r > 0 else 0  # diagonal q-trim
                nc.tensor.matmul(
                    slot[:, idx * 512 + t:(idx + 1) * 512],
                    kt_src[:, bass.ts(ki, 128)],
                    qt_src[:, j * 512 + t:(j + 1) * 512],
                    start=True, stop=True)
            return slot

        def emit_exp(j, eb_t, grp, slot):
            ncols = 512 * len(grp)
            r0 = grp[0] - 4 * j
            t0 = 128 * r0 if r0 > 0 else 0  # columns before t0 are never read
            nc.scalar.activation(
                eb_t[:, grp[0] * 512 + t0:grp[0] * 512 + ncols],
                slot[:, t0:ncols], AF.Exp, scale=0.125)
            for ki in grp:
                r = ki - 4 * j
                if r >= 0:
                    # causal mask inside the diagonal 128x128 block
                    blk = eb_t[:, ki * 512 + 128 * r:ki * 512 + 128 * r + 128]
                    nc.gpsimd.affine_select(
                        blk, blk, pattern=[[1, 128]],
                        compare_op=mybir.AluOpType.is_ge, fill=0.0,
                        base=0, channel_multiplier=-1)

        pending = [None]  # (j, eb_t, grp, s0) carried across streams

        def emit_stream(j, h, eb_t):
            """Score+exp stream for (j, h): scores run one group ahead of
            exp (across stream boundaries too); deferred work fills the
            remaining PE time each round."""
            nk = 4 * j + 4
            # groups of 2 banks: 3 score groups in flight in the 6-bank
            # ring, deep enough to hide the exp write-ack latency
            groups = [list(range(g, min(g + 2, nk))) for g in range(0, nk, 2)]
            if sched_dbg:
                oldest = dq[0][2] if dq else -1
                print(f"stream j={j} h={h} gen={gen_state['cur']} "
                      f"qlen={len(dq)} qcost={dq_cost[0]:.0f} oldest_gen={oldest}")
            for gi, grp in enumerate(groups):
                s0 = emit_scores(j, h, grp)
                # exp immediately after its scores: the scheduler lowers the
                # exp's deps to "all PE work emitted so far", so nothing else
                # may sit between the scores and their exp
                emit_exp(j, eb_t, grp, s0)
                rounds_left = len(groups) - gi
                budget = max(500.0, min(900.0, dq_cost[0] / rounds_left))
                pop_budget(budget)
            gen_state["cur"] += 1

        def emit_attv(j, h, eb_t):
            """Queue att@v chunks + normalize (+ tails at h==2)."""
            jsl_base = j * 512

            def mk_chunk(qc, k0, k1, att_box, first, last):
                def clo():
                    if first:
                        att_box[0] = att_pool.tile([128, 65], f32, tag="att",
                                                   name="att")
                    att = att_box[0]
                    nkq = 4 * j + qc + 1
                    for ki in range(k0, k1):
                        nc.tensor.matmul(
                            att[:],
                            eb_t[:, ki * 512 + qc * 128:ki * 512 + qc * 128 + 128],
                            v_aug[:, ki * 195 + 65 * h:ki * 195 + 65 * h + 65],
                            start=(ki == 0), stop=(ki == nkq - 1))
                    if last:
                        rcp = rcp_pool.tile([128, 1], f32, tag="rcp",
                                            name="rcp")
                        nc.vector.reciprocal_approx_fast(out=rcp[:],
                                                         in_=att[:, 64:65])
                        if h == 0:
                            attn_t[qc] = attn_pool.tile([128, 192], bf16,
                                                        tag=f"attn{qc}",
                                                        name="attn")
                        nc.vector.tensor_scalar_mul(
                            attn_t[qc][:, h * 64:(h + 1) * 64], att[:, 0:64],
                            rcp[:])
                return clo

            for qc in range(4):
                nkq = 4 * j + qc + 1
                att_box = [None]
                # split long accumulations into <=16-matmul pieces
                k0 = 0
                while k0 < nkq:
                    k1 = min(k0 + 16, nkq)
                    push(27 * (k1 - k0) + (100 if k1 == nkq else 0),
                         mk_chunk(qc, k0, k1, att_box, k0 == 0, k1 == nkq))
                    k0 = k1

            if h == 2:
                ao_box = [None, None]
                for qc in range(4):
                    push(450, mk_transpose(j, qc, ao_box))
                for qc in range(4):
                    for ns in range(2):
                        push(550, mk_proj(j, qc, ns, ao_box))

        def mk_transpose(j, qc, ao_box):
            def clo():
                if qc == 0:
                    ao_box[0] = aoT_pool.tile([128, 512], bf16, tag="ao01",
                                              name="ao01")
                    ao_box[1] = aoT_pool.tile([64, 512], bf16, tag="ao2",
                                              name="ao2")
                tps = next_slot().bitcast(bf16)
                tp1 = tps[:, 0:1024]
                tp2 = tps[:, 1024:2048]
                nc.tensor.transpose(tp1[:, 0:128], attn_t[qc][:, 0:128],
                                    ident[:])
                nc.tensor.transpose(tp2[0:64, 0:128], attn_t[qc][:, 128:192],
                                    ident[:])
                csl = slice(qc * 128, qc * 128 + 128)
                nc.vector.tensor_copy(ao_box[0][:, csl], tp1[:, 0:128])
                nc.vector.tensor_copy(ao_box[1][:, csl], tp2[0:64, 0:128])
            return clo

        def mk_proj(j, qc, ns, ao_box):
            def clo():
                msl = slice(j * 512 + qc * 128, j * 512 + qc * 128 + 128)
                csl = slice(qc * 128, qc * 128 + 128)
                py = next_slot()[:, 0:384]
                nc.tensor.matmul(py, ao_box[0][:, csl],
                                 wp01_sb[:, ns * 384:(ns + 1) * 384],
                                 start=True, stop=False)
                nc.tensor.matmul(py, ao_box[1][:, csl],
                                 wp2_sb[:, ns * 384:(ns + 1) * 384],
                                 start=False, stop=True)
                y_sb = y_pool.tile([128, 384], f32, tag="y", name="ysb")
                nc.vector.tensor_copy(y_sb[:], py)
                nc.sync.dma_start(y_d[msl, ns * 384:(ns + 1) * 384], y_sb[:])
            return clo

        # ------------- main pipeline -------------
        attn_t = [None] * 4  # per-q-chunk staging tiles (rebound per j)
        _, qk0, v0 = emit_qkv(0, xt_pre=xt0)
        for _, fn in qk0[:4]:
            fn()  # bootstrap: q01/k01 chains only
        for item in qk0[4:] + v0:
            push(*item)  # slot2 + v hide inside stream (0,0)
        parts = {}
        if NT > 1:
            parts[1] = emit_qkv(1)
            for item in parts[1][0]:
                item[1]()  # xt prefetch for j=1 up front
        for j in range(NT):
            for h in range(HPC):
                eb_t = eb_pool.tile([128, (NT * 4) * 512], bf16, tag="eb",
                                    name="eb")
                if h == 0 and j + 1 < NT:
                    for item in parts[j + 1][1]:
                        push(*item)
                elif h == 1 and j + 1 < NT:
                    for item in parts[j + 1][2]:
                        push(*item)
                elif h == 2 and j + 2 < NT:
                    parts[j + 2] = emit_qkv(j + 2)
                    for item in parts[j + 2][0]:
                        push(*item)  # xt prefetch two tiles ahead
                emit_stream(j, h, eb_t)
                emit_attv(j, h, eb_t)
        pop_budget(float("inf"))

    nc.compile()
    return nc


_NC_CACHE = {}


def _get_nc(T):
    if T not in _NC_CACHE:
        _NC_CACHE[T] = build_nc(T)
    return _NC_CACHE[T]


def make_core_inputs(x, W_attn, b_attn, W_proj):
    """Host-side prep: per-core input dicts (see module docstring)."""
    import ml_dtypes
    B, T, _ = x.shape
    xts = [np.ascontiguousarray(x[b].T).astype(ml_dtypes.bfloat16)
           for b in range(B)]
    in_maps = []
    for core in range(N_CORES):
        b = core // (N_CORES // B)
        h0 = HPC * (core % (N_CORES // B))
        c01 = slice(h0 * D, (h0 + 2) * D)      # first two heads
        c2 = slice((h0 + 2) * D, (h0 + 3) * D)  # third head
        # reference splits qkv as (k, q, v): k cols 0:C, q cols C:2C, v 2C:3C
        q01 = W_attn[:, C:2 * C][:, c01]
        k01 = W_attn[:, 0:C][:, c01]
        q2 = W_attn[:, C:2 * C][:, c2]
        k2 = W_attn[:, 0:C][:, c2]
        wv = W_attn[:, 2 * C:3 * C][:, h0 * D:(h0 + 3) * D]
        wqv = np.concatenate([q01, k01, q2, k2, wv], axis=1)
        bqk = np.zeros((128, 3), np.float32)
        bqk[:, 0] = b_attn[C:2 * C][c01]
        bqk[:, 1] = b_attn[0:C][c01]
        bqk[0:64, 2] = b_attn[C:2 * C][c2]
        bqk[64:128, 2] = b_attn[0:C][c2]
        bv = b_attn[2 * C:3 * C][h0 * D:(h0 + 3) * D].reshape(1, 192)
        wp01 = W_proj[h0 * D:(h0 + 2) * D, :]
        wp2 = W_proj[(h0 + 2) * D:(h0 + 3) * D, :]
        in_maps.append({
            "xt": xts[b],
            "wqv": np.ascontiguousarray(wqv).astype(ml_dtypes.bfloat16),
            "bv": np.ascontiguousarray(bv).astype(ml_dtypes.bfloat16),
            "bqk": bqk,
            "wp01": np.ascontiguousarray(wp01).astype(ml_dtypes.bfloat16),
            "wp2": np.ascontiguousarray(wp2).astype(ml_dtypes.bfloat16),
        })
    return in_maps


def kernel(x, W_attn, b_attn, W_proj, b_proj):
    x = np.asarray(x, dtype=np.float32)
    W_attn = np.asarray(W_attn, dtype=np.float32)
    b_attn = np.asarray(b_attn, dtype=np.float32)
    W_proj = np.asarray(W_proj, dtype=np.float32)
    b_proj = np.asarray(b_proj, dtype=np.float32)
    B, T, _ = x.shape

    nc = _get_nc(T)
    in_maps = make_core_inputs(x, W_attn, b_attn, W_proj)
    res = None
    for attempt in range(3):
        try:
            res = run_bass_kernel_spmd(nc, in_maps, list(range(N_CORES)))
            break
        except Exception:
            # transient NRT_EXEC_UNIT_UNRECOVERABLE has been observed once
            # after a prior crashed process; a retry succeeds
            if attempt == 2:
                raise
    global LAST_RUN
    LAST_RUN = res

    gpb = N_CORES // B
    out = np.empty((B, T, C), np.float32)
    for b in range(B):
        acc = res.results[b * gpb]["y"].astype(np.float32)
        for g in range(1, gpb):
            acc = acc + res.results[b * gpb + g]["y"]
        out[b] = acc + b_proj[None, :]
    return out


# revision 48
# speedup vs baseline: 1.3930x; 1.0340x over previous
"""Causal self-attention (B=2, T=4096, C=768, H=12) on 8 TRN2 NeuronCores.

Sharding: batch x head-group. Core c handles batch b=c//4 and heads
h0..h0+2 where h0 = 3*(c%4). Each core computes the qkv projection for
its 3 heads, full causal attention, and a partial output projection; the
host sums the 4 partials per batch and adds the projection bias.

v2 design (all matmul inputs bf16):
- q/k are produced transposed ([d, T]); v is produced directly in
  natural layout [k, d] per 128-k-tile (lhsT = x^T tile, rhs = Wv
  chunk), with the v bias folded in via a ones-row matmul. No v
  transposes.
- Scores keep the [k-part, q-free] orientation; exp runs on the scalar
  engine over groups of up to 3 PSUM banks into SBUF bf16 tiles (eb).
  Above-diagonal q columns are trimmed from the score matmuls (exp of
  the resulting zeroed columns is never consumed).
- att@v is reoriented to out [q-part, 65]: lhsT = eb column block,
  rhs = v_aug [k, 65] (v plus a ones column). The 65th output column
  is the softmax denominator, per-partition, so normalization is a
  reciprocal + tensor_scalar multiply - no partition broadcast.
- The normalized attention output [q, d] is transposed back (PE, bf16)
  and packed into aoT01 [128, T] / aoT2 [64, T] so the output
  projection contracts 128+64 at a time.
- PSUM: a pool of three 2-bank "slot" tiles serves scores (pairs of
  k-tiles, exp'd as one [128,<=1024] activation), qkv slots, transposes
  and proj outputs; 2 banks hold att accumulators. Pool-tile rotation
  gives exact per-slot WAR/RAW deps - slicing one big PSUM tile instead
  serializes everything (PSUM dep tracking is coarse).
- Scheduling: exp on the Act engine (~220us busy) and matmul rows on
  PE (~202us) are the co-bottlenecks. The tile scheduler lowers each
  exp's dependencies to "all PE work emitted so far", so each score
  pair is immediately followed by its exp, and all other PE work
  (qkv, att@v, transposes, proj) is queued as (cost, closure) items
  popped between rounds against a budget fitted to each exp's
  duration. qkv work is a priority queue (it releases x-tile buffers
  and feeds the next stream's scores). DVE does the PSUM->SBUF
  copies/normalize (GPSIMD cannot read PSUM on real HW); Pool does
  the causal masks (affine_select on eb) and some weight-load DMAs.
  Head h's att@v+normalize work drains during head h+1's score/exp
  stream; eb tiles (bf16 exp outputs) are triple-buffered in SBUF.
"""

import sys

for _p in ("/opt/trn_rl_repo",):
    if _p not in sys.path:
        sys.path.insert(0, _p)

from collections import deque
from contextlib import ExitStack

import numpy as np

import concourse.bass as bass  # noqa: F401
import concourse.mybir as mybir
import concourse.tile as tile
from concourse import bacc
from concourse.bass_utils import run_bass_kernel_spmd
from concourse.masks import make_identity

f32 = mybir.dt.float32
bf16 = mybir.dt.bfloat16
AF = mybir.ActivationFunctionType

C = 768
D = 64
N_HEAD = 12
HPC = 3  # heads per core
N_CORES = 8


def build_nc(T):
    NT = T // 512  # q tiles
    KT = T // 128  # k tiles
    CK = C // 128  # contraction chunks

    nc = bacc.Bacc("TRN2", target_bir_lowering=False, debug=False,
                   num_devices=N_CORES)
    xt_d = nc.dram_tensor("xt", [C, T], bf16, kind="ExternalInput").ap()
    wqv_d = nc.dram_tensor("wqv", [C, 576], bf16, kind="ExternalInput").ap()
    bv_d = nc.dram_tensor("bv", [1, 192], bf16, kind="ExternalInput").ap()
    bqk_d = nc.dram_tensor("bqk", [128, 3], f32, kind="ExternalInput").ap()
    wp01_d = nc.dram_tensor("wp01", [128, C], bf16, kind="ExternalInput").ap()
    wp2_d = nc.dram_tensor("wp2", [64, C], bf16, kind="ExternalInput").ap()
    y_d = nc.dram_tensor("y", [T, C], f32, kind="ExternalOutput").ap()

    with tile.TileContext(nc) as tc, ExitStack() as ctx:
        sb = ctx.enter_context(tc.tile_pool(name="sb", bufs=1))

        # ---- persistent SBUF ----
        wqv_sb = [sb.tile([128, 576], bf16, tag=f"wqv{c}", name=f"wqv{c}")
                  for c in range(CK)]
        bv_sb = sb.tile([1, 192], bf16, tag="bv")
        bqk_sb = sb.tile([128, 3], f32, tag="bqk")
        wp01_sb = sb.tile([128, C], bf16, tag="wp01")
        wp2_sb = sb.tile([64, C], bf16, tag="wp2")
        ident = sb.tile([128, 128], bf16, tag="ident")
        ones1 = sb.tile([1, 128], bf16, tag="ones1")
        qT_AB = sb.tile([128, T], bf16, tag="qAB")
        kT_AB = sb.tile([128, T], bf16, tag="kAB")
        qT_C = sb.tile([64, T], bf16, tag="qC")
        kT_C = sb.tile([128, T], bf16, tag="kC")
        v_aug = sb.tile([128, KT * 195], bf16, tag="vaug")
        aoT_pool = ctx.enter_context(tc.tile_pool(name="aotp", bufs=3))

        # ---- pools ----
        xt_pool = ctx.enter_context(tc.tile_pool(name="xtp", bufs=13))
        y_pool = ctx.enter_context(tc.tile_pool(name="yp", bufs=6))
        eb_pool = ctx.enter_context(tc.tile_pool(name="ebp", bufs=3))
        attn_pool = ctx.enter_context(tc.tile_pool(name="attnp", bufs=4))
        rcp_pool = ctx.enter_context(tc.tile_pool(name="rcpp", bufs=6))
        ring_pool = ctx.enter_context(
            tc.tile_pool(name="ringp", bufs=2, space="PSUM"))
        att_pool = ctx.enter_context(
            tc.tile_pool(name="attp", bufs=2, space="PSUM"))

        # startup: x tile DMAs for j=0 first, then weights in use order
        xt0 = [xt_pool.tile([128, 512], bf16, tag="xt", name="xt0")
               for _ in range(CK)]
        for c in range(CK):
            nc.sync.dma_start(xt0[c][:], xt_d[c * 128:(c + 1) * 128, 0:512])
            # weight chunks split across the Act and Pool DMA queues so the
            # first qkv chains start ~6us earlier
            if c < 3:
                nc.scalar.dma_start(out=wqv_sb[c][:],
                                    in_=wqv_d[c * 128:(c + 1) * 128, :])
            else:
                nc.gpsimd.dma_start(out=wqv_sb[c][:],
                                    in_=wqv_d[c * 128:(c + 1) * 128, :])
        nc.sync.dma_start(bqk_sb[:], bqk_d)
        nc.sync.dma_start(bv_sb[:], bv_d)
        nc.sync.dma_start(wp01_sb[:], wp01_d)
        nc.sync.dma_start(wp2_sb[:], wp2_d)
        make_identity(nc, ident[:])
        nc.vector.memset(ones1[:], 1.0)
        va_r = v_aug[:].rearrange("p (k c) -> p k c", c=195)
        for h in range(HPC):
            nc.gpsimd.memset(va_r[:, :, 65 * h + 64:65 * h + 65], 1.0)

        def next_slot():
            """Claim a 2-bank PSUM slot; pool rotation provides exact
            per-slot WAR/RAW dependencies."""
            return ring_pool.tile([128, 1536], f32, tag="ring", name="slot")

        # ------------- deferred-work queue -------------
        # items are (pe_cost_ns, closure); popped between score groups
        # against a per-round budget so the next score group is never far
        # behind in the in-order PE queue.
        import os
        sched_dbg = os.environ.get("KDBG_SCHED") == "1"
        dq = deque()
        uq = deque()  # urgent: qkv work (releases xt tiles, feeds scores)
        dq_cost = [0.0]
        gen_state = {"cur": 0}

        def push(cost, fn):
            dq.append((cost, fn, gen_state["cur"]))
            dq_cost[0] += cost

        def push_urgent(cost, fn):
            uq.append((cost, fn))
            dq_cost[0] += cost

        def pop_budget(budget):
            spent = 0.0
            while uq and spent < max(budget, 900.0):
                cost, fn = uq.popleft()
                dq_cost[0] -= cost
                fn()
                spent += cost
            while dq and spent < budget:
                cost, fn, g = dq.popleft()
                dq_cost[0] -= cost
                fn()
                spent += cost

        # ------------- emission helpers -------------
        def emit_qkv(j, xt_pre=None):
            """Queue q/k/v production for q-tile j."""
            jsl = bass.ts(j, 512)
            xt_t = list(xt_pre) if xt_pre is not None else [None] * CK

            def clo_dma():
                for c in range(CK):
                    xt_t[c] = xt_pool.tile([128, 512], bf16, tag="xt",
                                           name="xt")
                    nc.sync.dma_start(
                        xt_t[c][:],
                        xt_d[c * 128:(c + 1) * 128, j * 512:(j + 1) * 512])

            def mk_qk(s):
                state = {}

                def clo_a():
                    state["sl"] = next_slot()
                    out = state["sl"][:, 0:512]
                    for c in range(3):
                        nc.tensor.matmul(out,
                                         wqv_sb[c][:, s * 128:(s + 1) * 128],
                                         xt_t[c][:], start=(c == 0),
                                         stop=False)

                def clo_b():
                    out = state["sl"][:, 0:512]
                    for c in range(3, CK):
                        nc.tensor.matmul(out,
                                         wqv_sb[c][:, s * 128:(s + 1) * 128],
                                         xt_t[c][:], start=False,
                                         stop=(c == CK - 1))
                    if s == 0:
                        nc.vector.tensor_scalar_add(qT_AB[:, jsl], out,
                                                    bqk_sb[:, 0:1])
                    elif s == 1:
                        nc.vector.tensor_scalar_add(kT_AB[:, jsl], out,
                                                    bqk_sb[:, 1:2])
                    else:
                        nc.vector.tensor_scalar_add(qT_C[:, jsl],
                                                    out[0:64, :],
                                                    bqk_sb[0:64, 2:3])
                        nc.vector.tensor_scalar_add(kT_C[64:128, jsl],
                                                    out[64:128, :],
                                                    bqk_sb[64:128, 2:3])
                        # k2 must also live on partitions 0:64 (score lhsT)
                        nc.sync.dma_start(kT_C[0:64, jsl], kT_C[64:128, jsl])
                return clo_a, clo_b

            def mk_v(kt):
                def clo():
                    ki = 4 * j + kt
                    out = next_slot()[:, 0:192]
                    for c in range(CK):
                        nc.tensor.matmul(
                            out, xt_t[c][:, kt * 128:(kt + 1) * 128],
                            wqv_sb[c][:, 384:576], start=(c == 0), stop=False)
                    nc.tensor.matmul(out, ones1[:], bv_sb[:],
                                     start=False, stop=True)
                    dst = va_r[:, ki:ki + 1, :].rearrange(
                        "p k (h c) -> p (k h) c", c=65)[:, :, 0:64]
                    src = out.rearrange("p (h c) -> p h c", c=64)
                    nc.vector.tensor_copy(dst, src)
                return clo

            qk_part = []
            for s in range(3):
                ca, cb = mk_qk(s)
                qk_part.append((640, ca))
                qk_part.append((640, cb))
            v_part = [(600, mk_v(kt)) for kt in range(4)]
            dma_part = [] if xt_pre is not None else [(50, clo_dma)]
            return dma_part, qk_part, v_part

        def head_qk(h):
            if h == 0:
                return kT_AB[0:64, :], qT_AB[0:64, :]
            if h == 1:
                return kT_AB[64:128, :], qT_AB[64:128, :]
            return kT_C[0:64, :], qT_C[:]

        def emit_scores(j, h, grp):
            kt_src, qt_src = head_qk(h)
            slot = next_slot()
            for idx, ki in enumerate(grp):
                r = ki - 4 * j
                t = 128 * r if r > 0 else 0  # diagonal q-trim
                nc.tensor.matmul(
                    slot[:, idx * 512 + t:(idx + 1) * 512],
                    kt_src[:, bass.ts(ki, 128)],
                    qt_src[:, j * 512 + t:(j + 1) * 512],
                    start=True, stop=True)
            return slot

        def emit_exp(j, eb_t, grp, slot):
            ncols = 512 * len(grp)
            r0 = grp[0] - 4 * j
            t0 = 128 * r0 if r0 > 0 else 0  # columns before t0 are never read
            nc.scalar.activation(
                eb_t[:, grp[0] * 512 + t0:grp[0] * 512 + ncols],
                slot[:, t0:ncols], AF.Exp, scale=0.125)
            for ki in grp:
                r = ki - 4 * j
                if r >= 0:
                    # causal mask inside the diagonal 128x128 block
                    blk = eb_t[:, ki * 512 + 128 * r:ki * 512 + 128 * r + 128]
                    nc.gpsimd.affine_select(
                        blk, blk, pattern=[[1, 128]],
                        compare_op=mybir.AluOpType.is_ge, fill=0.0,
                        base=0, channel_multiplier=-1)

        pending = [None]  # (j, eb_t, grp, s0) carried across streams

        def emit_stream(j, h, eb_t, tail=None):
            """Score+exp stream for (j, h): scores run one group ahead of
            exp (across stream boundaries too); deferred work fills the
            remaining PE time each round."""
            nk = 4 * j + 4
            # groups of 2 banks: 3 score groups in flight in the 6-bank
            # ring, deep enough to hide the exp write-ack latency
            groups = [list(range(g, min(g + 3, nk))) for g in range(0, nk, 3)]
            if sched_dbg:
                oldest = dq[0][2] if dq else -1
                print(f"stream j={j} h={h} gen={gen_state['cur']} "
                      f"qlen={len(dq)} qcost={dq_cost[0]:.0f} oldest_gen={oldest}")
            for gi, grp in enumerate(groups):
                s0 = emit_scores(j, h, grp)
                # exp immediately after its scores: the scheduler lowers the
                # exp's deps to "all PE work emitted so far", so nothing else
                # may sit between the scores and their exp
                emit_exp(j, eb_t, grp, s0)
                rounds_left = len(groups) - gi
                budget = max(500.0, min(900.0, dq_cost[0] / rounds_left))
                pop_budget(budget)
            gen_state["cur"] += 1

        def emit_attv(j, h, eb_t, sink=None):
            """Queue att@v chunks + normalize (+ tails at h==2)."""
            sink = sink if sink is not None else push
            jsl_base = j * 512

            def mk_chunk(qc, k0, k1, att_box, first, last):
                def clo():
                    if first:
                        att_box[0] = att_pool.tile([128, 65], f32, tag="att",
                                                   name="att")
                    att = att_box[0]
                    nkq = 4 * j + qc + 1
                    for ki in range(k0, k1):
                        nc.tensor.matmul(
                            att[:],
                            eb_t[:, ki * 512 + qc * 128:ki * 512 + qc * 128 + 128],
                            v_aug[:, ki * 195 + 65 * h:ki * 195 + 65 * h + 65],
                            start=(ki == 0), stop=(ki == nkq - 1))
                    if last:
                        rcp = rcp_pool.tile([128, 1], f32, tag="rcp",
                                            name="rcp")
                        nc.vector.reciprocal_approx_fast(out=rcp[:],
                                                         in_=att[:, 64:65])
                        if h == 0:
                            attn_t[qc] = attn_pool.tile([128, 192], bf16,
                                                        tag=f"attn{qc}",
                                                        name="attn")
                        nc.vector.tensor_scalar_mul(
                            attn_t[qc][:, h * 64:(h + 1) * 64], att[:, 0:64],
                            rcp[:])
                return clo

            for qc in range(4):
                nkq = 4 * j + qc + 1
                att_box = [None]
                # split long accumulations into <=16-matmul pieces
                k0 = 0
                while k0 < nkq:
                    k1 = min(k0 + 16, nkq)
                    sink(27 * (k1 - k0) + (100 if k1 == nkq else 0),
                         mk_chunk(qc, k0, k1, att_box, k0 == 0, k1 == nkq))
                    k0 = k1

            if h == 2:
                ao_box = [None, None]
                for qc in range(4):
                    sink(450, mk_transpose(j, qc, ao_box))
                for qc in range(4):
                    for ns in range(2):
                        sink(550, mk_proj(j, qc, ns, ao_box))

        def mk_transpose(j, qc, ao_box):
            def clo():
                if qc == 0:
                    ao_box[0] = aoT_pool.tile([128, 512], bf16, tag="ao01",
                                              name="ao01")
                    ao_box[1] = aoT_pool.tile([64, 512], bf16, tag="ao2",
                                              name="ao2")
                tps = next_slot().bitcast(bf16)
                tp1 = tps[:, 0:1024]
                tp2 = tps[:, 1024:2048]
                nc.tensor.transpose(tp1[:, 0:128], attn_t[qc][:, 0:128],
                                    ident[:])
                nc.tensor.transpose(tp2[0:64, 0:128], attn_t[qc][:, 128:192],
                                    ident[:])
                csl = slice(qc * 128, qc * 128 + 128)
                nc.vector.tensor_copy(ao_box[0][:, csl], tp1[:, 0:128])
                nc.vector.tensor_copy(ao_box[1][:, csl], tp2[0:64, 0:128])
            return clo

        def mk_proj(j, qc, ns, ao_box):
            def clo():
                msl = slice(j * 512 + qc * 128, j * 512 + qc * 128 + 128)
                csl = slice(qc * 128, qc * 128 + 128)
                py = next_slot()[:, 0:384]
                nc.tensor.matmul(py, ao_box[0][:, csl],
                                 wp01_sb[:, ns * 384:(ns + 1) * 384],
                                 start=True, stop=False)
                nc.tensor.matmul(py, ao_box[1][:, csl],
                                 wp2_sb[:, ns * 384:(ns + 1) * 384],
                                 start=False, stop=True)
                y_sb = y_pool.tile([128, 384], f32, tag="y", name="ysb")
                nc.vector.tensor_copy(y_sb[:], py)
                nc.sync.dma_start(y_d[msl, ns * 384:(ns + 1) * 384], y_sb[:])
            return clo

        # ------------- main pipeline -------------
        attn_t = [None] * 4  # per-q-chunk staging tiles (rebound per j)
        _, qk0, v0 = emit_qkv(0, xt_pre=xt0)
        for _, fn in qk0[:4]:
            fn()  # bootstrap: q01/k01 chains only
        for item in qk0[4:] + v0:
            push_urgent(*item)  # slot2 + v hide inside stream (0,0)
        parts = {}
        if NT > 1:
            parts[1] = emit_qkv(1)
            for item in parts[1][0]:
                item[1]()  # xt prefetch for j=1 up front
        for j in range(NT):
            for h in range(HPC):
                eb_t = eb_pool.tile([128, (NT * 4) * 512], bf16, tag="eb",
                                    name="eb")
                if h == 0 and j + 1 < NT:
                    for item in parts[j + 1][1]:
                        push_urgent(*item)
                elif h == 1 and j + 1 < NT:
                    for item in parts[j + 1][2]:
                        push_urgent(*item)
                elif h == 2 and j + 2 < NT:
                    parts[j + 2] = emit_qkv(j + 2)
                    for item in parts[j + 2][0]:
                        push_urgent(*item)  # xt prefetch two tiles ahead
                last = (j == NT - 1 and h == 2)
                emit_stream(j, h, eb_t,
                            tail=(lambda: emit_attv(j, h, eb_t,
                                                    sink=lambda c, f: f()))
                            if last else None)
                if not last:
                    emit_attv(j, h, eb_t)
        pop_budget(float("inf"))

    nc.compile()
    return nc


_NC_CACHE = {}


def _get_nc(T):
    if T not in _NC_CACHE:
        _NC_CACHE[T] = build_nc(T)
    return _NC_CACHE[T]


def make_core_inputs(x, W_attn, b_attn, W_proj):
    """Host-side prep: per-core input dicts (see module docstring)."""
    import ml_dtypes
    B, T, _ = x.shape
    xts = [np.ascontiguousarray(x[b].T).astype(ml_dtypes.bfloat16)
           for b in range(B)]
    in_maps = []
    for core in range(N_CORES):
        b = core // (N_CORES // B)
        h0 = HPC * (core % (N_CORES // B))
        c01 = slice(h0 * D, (h0 + 2) * D)      # first two heads
        c2 = slice((h0 + 2) * D, (h0 + 3) * D)  # third head
        # reference splits qkv as (k, q, v): k cols 0:C, q cols C:2C, v 2C:3C
        q01 = W_attn[:, C:2 * C][:, c01]
        k01 = W_attn[:, 0:C][:, c01]
        q2 = W_attn[:, C:2 * C][:, c2]
        k2 = W_attn[:, 0:C][:, c2]
        wv = W_attn[:, 2 * C:3 * C][:, h0 * D:(h0 + 3) * D]
        wqv = np.concatenate([q01, k01, q2, k2, wv], axis=1)
        bqk = np.zeros((128, 3), np.float32)
        bqk[:, 0] = b_attn[C:2 * C][c01]
        bqk[:, 1] = b_attn[0:C][c01]
        bqk[0:64, 2] = b_attn[C:2 * C][c2]
        bqk[64:128, 2] = b_attn[0:C][c2]
        bv = b_attn[2 * C:3 * C][h0 * D:(h0 + 3) * D].reshape(1, 192)
        wp01 = W_proj[h0 * D:(h0 + 2) * D, :]
        wp2 = W_proj[(h0 + 2) * D:(h0 + 3) * D, :]
        in_maps.append({
            "xt": xts[b],
            "wqv": np.ascontiguousarray(wqv).astype(ml_dtypes.bfloat16),
            "bv": np.ascontiguousarray(bv).astype(ml_dtypes.bfloat16),
            "bqk": bqk,
            "wp01": np.ascontiguousarray(wp01).astype(ml_dtypes.bfloat16),
            "wp2": np.ascontiguousarray(wp2).astype(ml_dtypes.bfloat16),
        })
    return in_maps


def kernel(x, W_attn, b_attn, W_proj, b_proj):
    x = np.asarray(x, dtype=np.float32)
    W_attn = np.asarray(W_attn, dtype=np.float32)
    b_attn = np.asarray(b_attn, dtype=np.float32)
    W_proj = np.asarray(W_proj, dtype=np.float32)
    b_proj = np.asarray(b_proj, dtype=np.float32)
    B, T, _ = x.shape

    nc = _get_nc(T)
    in_maps = make_core_inputs(x, W_attn, b_attn, W_proj)
    res = None
    for attempt in range(3):
        try:
            res = run_bass_kernel_spmd(nc, in_maps, list(range(N_CORES)))
            break
        except Exception:
            # transient NRT_EXEC_UNIT_UNRECOVERABLE has been observed once
            # after a prior crashed process; a retry succeeds
            if attempt == 2:
                raise
    global LAST_RUN
    LAST_RUN = res

    gpb = N_CORES // B
    out = np.empty((B, T, C), np.float32)
    for b in range(B):
        acc = res.results[b * gpb]["y"].astype(np.float32)
        for g in range(1, gpb):
            acc = acc + res.results[b * gpb + g]["y"]
        out[b] = acc + b_proj[None, :]
    return out


# revision 49
# speedup vs baseline: 1.3944x; 1.0010x over previous
"""Causal self-attention (B=2, T=4096, C=768, H=12) on 8 TRN2 NeuronCores.

Sharding: batch x head-group. Core c handles batch b=c//4 and heads
h0..h0+2 where h0 = 3*(c%4). Each core computes the qkv projection for
its 3 heads, full causal attention, and a partial output projection; the
host sums the 4 partials per batch and adds the projection bias.

v2 design (all matmul inputs bf16):
- q/k are produced transposed ([d, T]); v is produced directly in
  natural layout [k, d] per 128-k-tile (lhsT = x^T tile, rhs = Wv
  chunk), with the v bias folded in via a ones-row matmul. No v
  transposes.
- Scores keep the [k-part, q-free] orientation; exp runs on the scalar
  engine over groups of up to 3 PSUM banks into SBUF bf16 tiles (eb).
  Above-diagonal q columns are trimmed from the score matmuls (exp of
  the resulting zeroed columns is never consumed).
- att@v is reoriented to out [q-part, 65]: lhsT = eb column block,
  rhs = v_aug [k, 65] (v plus a ones column). The 65th output column
  is the softmax denominator, per-partition, so normalization is a
  reciprocal + tensor_scalar multiply - no partition broadcast.
- The normalized attention output [q, d] is transposed back (PE, bf16)
  and packed into aoT01 [128, T] / aoT2 [64, T] so the output
  projection contracts 128+64 at a time.
- PSUM: a pool of three 2-bank "slot" tiles serves scores (pairs of
  k-tiles, exp'd as one [128,<=1024] activation), qkv slots, transposes
  and proj outputs; 2 banks hold att accumulators. Pool-tile rotation
  gives exact per-slot WAR/RAW deps - slicing one big PSUM tile instead
  serializes everything (PSUM dep tracking is coarse).
- Scheduling: exp on the Act engine (~220us busy) and matmul rows on
  PE (~202us) are the co-bottlenecks. The tile scheduler lowers each
  exp's dependencies to "all PE work emitted so far", so each score
  pair is immediately followed by its exp, and all other PE work
  (qkv, att@v, transposes, proj) is queued as (cost, closure) items
  popped between rounds against a budget fitted to each exp's
  duration. qkv work is a priority queue (it releases x-tile buffers
  and feeds the next stream's scores). DVE does the PSUM->SBUF
  copies/normalize (GPSIMD cannot read PSUM on real HW); Pool does
  the causal masks (affine_select on eb) and some weight-load DMAs.
  Head h's att@v+normalize work drains during head h+1's score/exp
  stream; eb tiles (bf16 exp outputs) are triple-buffered in SBUF.
"""

import sys

for _p in ("/opt/trn_rl_repo",):
    if _p not in sys.path:
        sys.path.insert(0, _p)

from collections import deque
from contextlib import ExitStack

import numpy as np

import concourse.bass as bass  # noqa: F401
import concourse.mybir as mybir
import concourse.tile as tile
from concourse import bacc
from concourse.bass_utils import run_bass_kernel_spmd
from concourse.masks import make_identity

f32 = mybir.dt.float32
bf16 = mybir.dt.bfloat16
AF = mybir.ActivationFunctionType

C = 768
D = 64
N_HEAD = 12
HPC = 3  # heads per core
N_CORES = 8


def build_nc(T):
    NT = T // 512  # q tiles
    KT = T // 128  # k tiles
    CK = C // 128  # contraction chunks

    nc = bacc.Bacc("TRN2", target_bir_lowering=False, debug=False,
                   num_devices=N_CORES)
    xt_d = nc.dram_tensor("xt", [C, T], bf16, kind="ExternalInput").ap()
    wqv_d = nc.dram_tensor("wqv", [C, 576], bf16, kind="ExternalInput").ap()
    bv_d = nc.dram_tensor("bv", [1, 192], bf16, kind="ExternalInput").ap()
    bqk_d = nc.dram_tensor("bqk", [128, 3], f32, kind="ExternalInput").ap()
    wp01_d = nc.dram_tensor("wp01", [128, C], bf16, kind="ExternalInput").ap()
    wp2_d = nc.dram_tensor("wp2", [64, C], bf16, kind="ExternalInput").ap()
    y_d = nc.dram_tensor("y", [T, C], f32, kind="ExternalOutput").ap()

    with tile.TileContext(nc) as tc, ExitStack() as ctx:
        sb = ctx.enter_context(tc.tile_pool(name="sb", bufs=1))

        # ---- persistent SBUF ----
        wqv_sb = [sb.tile([128, 576], bf16, tag=f"wqv{c}", name=f"wqv{c}")
                  for c in range(CK)]
        bv_sb = sb.tile([1, 192], bf16, tag="bv")
        bqk_sb = sb.tile([128, 3], f32, tag="bqk")
        wp01_sb = sb.tile([128, C], bf16, tag="wp01")
        wp2_sb = sb.tile([64, C], bf16, tag="wp2")
        ident = sb.tile([128, 128], bf16, tag="ident")
        ones1 = sb.tile([1, 128], bf16, tag="ones1")
        qT_AB = sb.tile([128, T], bf16, tag="qAB")
        kT_AB = sb.tile([128, T], bf16, tag="kAB")
        qT_C = sb.tile([64, T], bf16, tag="qC")
        kT_C = sb.tile([128, T], bf16, tag="kC")
        v_aug = sb.tile([128, KT * 195], bf16, tag="vaug")
        aoT_pool = ctx.enter_context(tc.tile_pool(name="aotp", bufs=3))

        # ---- pools ----
        xt_pool = ctx.enter_context(tc.tile_pool(name="xtp", bufs=13))
        y_pool = ctx.enter_context(tc.tile_pool(name="yp", bufs=6))
        eb_pool = ctx.enter_context(tc.tile_pool(name="ebp", bufs=3))
        attn_pool = ctx.enter_context(tc.tile_pool(name="attnp", bufs=4))
        rcp_pool = ctx.enter_context(tc.tile_pool(name="rcpp", bufs=6))
        ring_pool = ctx.enter_context(
            tc.tile_pool(name="ringp", bufs=2, space="PSUM"))
        att_pool = ctx.enter_context(
            tc.tile_pool(name="attp", bufs=2, space="PSUM"))

        # startup: x tile DMAs for j=0 first, then weights in use order
        xt0 = [xt_pool.tile([128, 512], bf16, tag="xt", name="xt0")
               for _ in range(CK)]
        for c in range(CK):
            nc.sync.dma_start(xt0[c][:], xt_d[c * 128:(c + 1) * 128, 0:512])
            # weight chunks split across the Act and Pool DMA queues so the
            # first qkv chains start ~6us earlier
            if c < 3:
                nc.scalar.dma_start(out=wqv_sb[c][:],
                                    in_=wqv_d[c * 128:(c + 1) * 128, :])
            else:
                nc.gpsimd.dma_start(out=wqv_sb[c][:],
                                    in_=wqv_d[c * 128:(c + 1) * 128, :])
        nc.sync.dma_start(bqk_sb[:], bqk_d)
        nc.sync.dma_start(bv_sb[:], bv_d)
        nc.sync.dma_start(wp01_sb[:], wp01_d)
        nc.sync.dma_start(wp2_sb[:], wp2_d)
        make_identity(nc, ident[:])
        nc.vector.memset(ones1[:], 1.0)
        va_r = v_aug[:].rearrange("p (k c) -> p k c", c=195)
        for h in range(HPC):
            nc.gpsimd.memset(va_r[:, :, 65 * h + 64:65 * h + 65], 1.0)

        def next_slot():
            """Claim a 2-bank PSUM slot; pool rotation provides exact
            per-slot WAR/RAW dependencies."""
            return ring_pool.tile([128, 1536], f32, tag="ring", name="slot")

        # ------------- deferred-work queue -------------
        # items are (pe_cost_ns, closure); popped between score groups
        # against a per-round budget so the next score group is never far
        # behind in the in-order PE queue.
        import os
        sched_dbg = os.environ.get("KDBG_SCHED") == "1"
        dq = deque()
        uq = deque()  # urgent: qkv work (releases xt tiles, feeds scores)
        dq_cost = [0.0]
        gen_state = {"cur": 0}

        def push(cost, fn):
            dq.append((cost, fn, gen_state["cur"]))
            dq_cost[0] += cost

        def push_urgent(cost, fn):
            uq.append((cost, fn))
            dq_cost[0] += cost

        def pop_budget(budget):
            spent = 0.0
            while uq and spent < max(budget, 900.0):
                cost, fn = uq.popleft()
                dq_cost[0] -= cost
                fn()
                spent += cost
            while dq and spent < budget:
                cost, fn, g = dq.popleft()
                dq_cost[0] -= cost
                fn()
                spent += cost

        # ------------- emission helpers -------------
        def emit_qkv(j, xt_pre=None):
            """Queue q/k/v production for q-tile j."""
            jsl = bass.ts(j, 512)
            xt_t = list(xt_pre) if xt_pre is not None else [None] * CK

            def clo_dma():
                for c in range(CK):
                    xt_t[c] = xt_pool.tile([128, 512], bf16, tag="xt",
                                           name="xt")
                    nc.sync.dma_start(
                        xt_t[c][:],
                        xt_d[c * 128:(c + 1) * 128, j * 512:(j + 1) * 512])

            def mk_qk(s):
                state = {}

                def clo_a():
                    state["sl"] = next_slot()
                    out = state["sl"][:, 0:512]
                    for c in range(3):
                        nc.tensor.matmul(out,
                                         wqv_sb[c][:, s * 128:(s + 1) * 128],
                                         xt_t[c][:], start=(c == 0),
                                         stop=False)

                def clo_b():
                    out = state["sl"][:, 0:512]
                    for c in range(3, CK):
                        nc.tensor.matmul(out,
                                         wqv_sb[c][:, s * 128:(s + 1) * 128],
                                         xt_t[c][:], start=False,
                                         stop=(c == CK - 1))
                    if s == 0:
                        nc.vector.tensor_scalar_add(qT_AB[:, jsl], out,
                                                    bqk_sb[:, 0:1])
                    elif s == 1:
                        nc.vector.tensor_scalar_add(kT_AB[:, jsl], out,
                                                    bqk_sb[:, 1:2])
                    else:
                        nc.vector.tensor_scalar_add(qT_C[:, jsl],
                                                    out[0:64, :],
                                                    bqk_sb[0:64, 2:3])
                        nc.vector.tensor_scalar_add(kT_C[64:128, jsl],
                                                    out[64:128, :],
                                                    bqk_sb[64:128, 2:3])
                        # k2 must also live on partitions 0:64 (score lhsT)
                        nc.sync.dma_start(kT_C[0:64, jsl], kT_C[64:128, jsl])
                return clo_a, clo_b

            def mk_v(kt):
                def clo():
                    ki = 4 * j + kt
                    out = next_slot()[:, 0:192]
                    for c in range(CK):
                        nc.tensor.matmul(
                            out, xt_t[c][:, kt * 128:(kt + 1) * 128],
                            wqv_sb[c][:, 384:576], start=(c == 0), stop=False)
                    nc.tensor.matmul(out, ones1[:], bv_sb[:],
                                     start=False, stop=True)
                    dst = va_r[:, ki:ki + 1, :].rearrange(
                        "p k (h c) -> p (k h) c", c=65)[:, :, 0:64]
                    src = out.rearrange("p (h c) -> p h c", c=64)
                    nc.vector.tensor_copy(dst, src)
                return clo

            qk_part = []
            for s in range(3):
                ca, cb = mk_qk(s)
                qk_part.append((640, ca))
                qk_part.append((640, cb))
            v_part = [(600, mk_v(kt)) for kt in range(4)]
            dma_part = [] if xt_pre is not None else [(50, clo_dma)]
            return dma_part, qk_part, v_part

        def head_qk(h):
            if h == 0:
                return kT_AB[0:64, :], qT_AB[0:64, :]
            if h == 1:
                return kT_AB[64:128, :], qT_AB[64:128, :]
            return kT_C[0:64, :], qT_C[:]

        def emit_scores(j, h, grp):
            kt_src, qt_src = head_qk(h)
            slot = next_slot()
            for idx, ki in enumerate(grp):
                r = ki - 4 * j
                t = 128 * r if r > 0 else 0  # diagonal q-trim
                nc.tensor.matmul(
                    slot[:, idx * 512 + t:(idx + 1) * 512],
                    kt_src[:, bass.ts(ki, 128)],
                    qt_src[:, j * 512 + t:(j + 1) * 512],
                    start=True, stop=True)
            return slot

        def emit_exp(j, eb_t, grp, slot):
            ncols = 512 * len(grp)
            r0 = grp[0] - 4 * j
            t0 = 128 * r0 if r0 > 0 else 0  # columns before t0 are never read
            nc.scalar.activation(
                eb_t[:, grp[0] * 512 + t0:grp[0] * 512 + ncols],
                slot[:, t0:ncols], AF.Exp, scale=0.125)
            for ki in grp:
                r = ki - 4 * j
                if r >= 0:
                    # causal mask inside the diagonal 128x128 block
                    blk = eb_t[:, ki * 512 + 128 * r:ki * 512 + 128 * r + 128]
                    nc.gpsimd.affine_select(
                        blk, blk, pattern=[[1, 128]],
                        compare_op=mybir.AluOpType.is_ge, fill=0.0,
                        base=0, channel_multiplier=-1)

        pending = [None]  # (j, eb_t, grp, s0) carried across streams

        def emit_stream(j, h, eb_t, tail=None, late=None):
            """Score+exp stream for (j, h): scores run one group ahead of
            exp (across stream boundaries too); deferred work fills the
            remaining PE time each round. `late` = (round_idx, [fns]) emitted
            right after that round (last-stream tail shortening)."""
            nk = 4 * j + 4
            # groups of 2 banks: 3 score groups in flight in the 6-bank
            # ring, deep enough to hide the exp write-ack latency
            groups = [list(range(g, min(g + 3, nk))) for g in range(0, nk, 3)]
            if sched_dbg:
                oldest = dq[0][2] if dq else -1
                print(f"stream j={j} h={h} gen={gen_state['cur']} "
                      f"qlen={len(dq)} qcost={dq_cost[0]:.0f} oldest_gen={oldest}")
            for gi, grp in enumerate(groups):
                s0 = emit_scores(j, h, grp)
                # exp immediately after its scores: the scheduler lowers the
                # exp's deps to "all PE work emitted so far", so nothing else
                # may sit between the scores and their exp
                emit_exp(j, eb_t, grp, s0)
                rounds_left = len(groups) - gi
                budget = max(500.0, min(900.0, dq_cost[0] / rounds_left))
                pop_budget(budget)
            gen_state["cur"] += 1

        def emit_attv(j, h, eb_t, sink=None):
            """Queue att@v chunks + normalize (+ tails at h==2)."""
            sink = sink if sink is not None else push
            jsl_base = j * 512

            def mk_chunk(qc, k0, k1, att_box, first, last):
                def clo():
                    if first:
                        att_box[0] = att_pool.tile([128, 65], f32, tag="att",
                                                   name="att")
                    att = att_box[0]
                    nkq = 4 * j + qc + 1
                    for ki in range(k0, k1):
                        nc.tensor.matmul(
                            att[:],
                            eb_t[:, ki * 512 + qc * 128:ki * 512 + qc * 128 + 128],
                            v_aug[:, ki * 195 + 65 * h:ki * 195 + 65 * h + 65],
                            start=(ki == 0), stop=(ki == nkq - 1))
                    if last:
                        rcp = rcp_pool.tile([128, 1], f32, tag="rcp",
                                            name="rcp")
                        nc.vector.reciprocal_approx_fast(out=rcp[:],
                                                         in_=att[:, 64:65])
                        if h == 0:
                            attn_t[qc] = attn_pool.tile([128, 192], bf16,
                                                        tag=f"attn{qc}",
                                                        name="attn")
                        nc.vector.tensor_scalar_mul(
                            attn_t[qc][:, h * 64:(h + 1) * 64], att[:, 0:64],
                            rcp[:])
                return clo

            for qc in range(4):
                nkq = 4 * j + qc + 1
                att_box = [None]
                # split long accumulations into <=16-matmul pieces
                k0 = 0
                while k0 < nkq:
                    k1 = min(k0 + 16, nkq)
                    sink(27 * (k1 - k0) + (100 if k1 == nkq else 0),
                         mk_chunk(qc, k0, k1, att_box, k0 == 0, k1 == nkq))
                    k0 = k1

            if h == 2:
                ao_box = [None, None]
                for qc in range(4):
                    sink(450, mk_transpose(j, qc, ao_box))
                for qc in range(4):
                    for ns in range(2):
                        sink(550, mk_proj(j, qc, ns, ao_box))

        def mk_transpose(j, qc, ao_box):
            def clo():
                if qc == 0:
                    ao_box[0] = aoT_pool.tile([128, 512], bf16, tag="ao01",
                                              name="ao01")
                    ao_box[1] = aoT_pool.tile([64, 512], bf16, tag="ao2",
                                              name="ao2")
                tps = next_slot().bitcast(bf16)
                tp1 = tps[:, 0:1024]
                tp2 = tps[:, 1024:2048]
                nc.tensor.transpose(tp1[:, 0:128], attn_t[qc][:, 0:128],
                                    ident[:])
                nc.tensor.transpose(tp2[0:64, 0:128], attn_t[qc][:, 128:192],
                                    ident[:])
                csl = slice(qc * 128, qc * 128 + 128)
                nc.vector.tensor_copy(ao_box[0][:, csl], tp1[:, 0:128])
                nc.vector.tensor_copy(ao_box[1][:, csl], tp2[0:64, 0:128])
            return clo

        def mk_proj(j, qc, ns, ao_box):
            def clo():
                msl = slice(j * 512 + qc * 128, j * 512 + qc * 128 + 128)
                csl = slice(qc * 128, qc * 128 + 128)
                py = next_slot()[:, 0:384]
                nc.tensor.matmul(py, ao_box[0][:, csl],
                                 wp01_sb[:, ns * 384:(ns + 1) * 384],
                                 start=True, stop=False)
                nc.tensor.matmul(py, ao_box[1][:, csl],
                                 wp2_sb[:, ns * 384:(ns + 1) * 384],
                                 start=False, stop=True)
                y_sb = y_pool.tile([128, 384], f32, tag="y", name="ysb")
                nc.vector.tensor_copy(y_sb[:], py)
                nc.sync.dma_start(y_d[msl, ns * 384:(ns + 1) * 384], y_sb[:])
            return clo

        # ------------- main pipeline -------------
        attn_t = [None] * 4  # per-q-chunk staging tiles (rebound per j)
        _, qk0, v0 = emit_qkv(0, xt_pre=xt0)
        for _, fn in qk0[:4]:
            fn()  # bootstrap: q01/k01 chains only
        for item in qk0[4:] + v0:
            push_urgent(*item)  # slot2 + v hide inside stream (0,0)
        parts = {}
        if NT > 1:
            parts[1] = emit_qkv(1)
            for item in parts[1][0]:
                item[1]()  # xt prefetch for j=1 up front
        for j in range(NT):
            for h in range(HPC):
                eb_t = eb_pool.tile([128, (NT * 4) * 512], bf16, tag="eb",
                                    name="eb")
                if h == 0 and j + 1 < NT:
                    for item in parts[j + 1][1]:
                        push_urgent(*item)
                elif h == 1 and j + 1 < NT:
                    for item in parts[j + 1][2]:
                        push_urgent(*item)
                elif h == 2 and j + 2 < NT:
                    parts[j + 2] = emit_qkv(j + 2)
                    for item in parts[j + 2][0]:
                        push_urgent(*item)  # xt prefetch two tiles ahead
                last = (j == NT - 1 and h == 2)
                if last:
                    # chunks 0/1 (and their tails) only need exps through
                    # round nk//2 - 2; emit them mid-stream so only chunks
                    # 2/3 trail the final exp
                    items = []
                    emit_attv(j, h, eb_t,
                              sink=lambda c, f: items.append(f))
                    early = items[0:4] + items[8:10] + items[12:16]
                    rest = [f for f in items if f not in early]
                    nrounds = (4 * j + 4 + 1) // 2
                    emit_stream(j, h, eb_t, tail=lambda: [f() for f in rest],
                                late=(nrounds - 2, early))
                else:
                    emit_stream(j, h, eb_t)
                    emit_attv(j, h, eb_t)
        pop_budget(float("inf"))

    nc.compile()
    return nc


_NC_CACHE = {}


def _get_nc(T):
    if T not in _NC_CACHE:
        _NC_CACHE[T] = build_nc(T)
    return _NC_CACHE[T]


def make_core_inputs(x, W_attn, b_attn, W_proj):
    """Host-side prep: per-core input dicts (see module docstring)."""
    import ml_dtypes
    B, T, _ = x.shape
    xts = [np.ascontiguousarray(x[b].T).astype(ml_dtypes.bfloat16)
           for b in range(B)]
    in_maps = []
    for core in range(N_CORES):
        b = core // (N_CORES // B)
        h0 = HPC * (core % (N_CORES // B))
        c01 = slice(h0 * D, (h0 + 2) * D)      # first two heads
        c2 = slice((h0 + 2) * D, (h0 + 3) * D)  # third head
        # reference splits qkv as (k, q, v): k cols 0:C, q cols C:2C, v 2C:3C
        q01 = W_attn[:, C:2 * C][:, c01]
        k01 = W_attn[:, 0:C][:, c01]
        q2 = W_attn[:, C:2 * C][:, c2]
        k2 = W_attn[:, 0:C][:, c2]
        wv = W_attn[:, 2 * C:3 * C][:, h0 * D:(h0 + 3) * D]
        wqv = np.concatenate([q01, k01, q2, k2, wv], axis=1)
        bqk = np.zeros((128, 3), np.float32)
        bqk[:, 0] = b_attn[C:2 * C][c01]
        bqk[:, 1] = b_attn[0:C][c01]
        bqk[0:64, 2] = b_attn[C:2 * C][c2]
        bqk[64:128, 2] = b_attn[0:C][c2]
        bv = b_attn[2 * C:3 * C][h0 * D:(h0 + 3) * D].reshape(1, 192)
        wp01 = W_proj[h0 * D:(h0 + 2) * D, :]
        wp2 = W_proj[(h0 + 2) * D:(h0 + 3) * D, :]
        in_maps.append({
            "xt": xts[b],
            "wqv": np.ascontiguousarray(wqv).astype(ml_dtypes.bfloat16),
            "bv": np.ascontiguousarray(bv).astype(ml_dtypes.bfloat16),
            "bqk": bqk,
            "wp01": np.ascontiguousarray(wp01).astype(ml_dtypes.bfloat16),
            "wp2": np.ascontiguousarray(wp2).astype(ml_dtypes.bfloat16),
        })
    return in_maps


def kernel(x, W_attn, b_attn, W_proj, b_proj):
    x = np.asarray(x, dtype=np.float32)
    W_attn = np.asarray(W_attn, dtype=np.float32)
    b_attn = np.asarray(b_attn, dtype=np.float32)
    W_proj = np.asarray(W_proj, dtype=np.float32)
    b_proj = np.asarray(b_proj, dtype=np.float32)
    B, T, _ = x.shape

    nc = _get_nc(T)
    in_maps = make_core_inputs(x, W_attn, b_attn, W_proj)
    res = None
    for attempt in range(3):
        try:
            res = run_bass_kernel_spmd(nc, in_maps, list(range(N_CORES)))
            break
        except Exception:
            # transient NRT_EXEC_UNIT_UNRECOVERABLE has been observed once
            # after a prior crashed process; a retry succeeds
            if attempt == 2:
                raise
    global LAST_RUN
    LAST_RUN = res

    gpb = N_CORES // B
    out = np.empty((B, T, C), np.float32)
    for b in range(B):
        acc = res.results[b * gpb]["y"].astype(np.float32)
        for g in range(1, gpb):
            acc = acc + res.results[b * gpb + g]["y"]
        out[b] = acc + b_proj[None, :]
    return out


# revision 50
# speedup vs baseline: 1.3966x; 1.0016x over previous
"""Causal self-attention (B=2, T=4096, C=768, H=12) on 8 TRN2 NeuronCores.

Sharding: batch x head-group. Core c handles batch b=c//4 and heads
h0..h0+2 where h0 = 3*(c%4). Each core computes the qkv projection for
its 3 heads, full causal attention, and a partial output projection; the
host sums the 4 partials per batch and adds the projection bias.

v2 design (all matmul inputs bf16):
- q/k are produced transposed ([d, T]); v is produced directly in
  natural layout [k, d] per 128-k-tile (lhsT = x^T tile, rhs = Wv
  chunk), with the v bias folded in via a ones-row matmul. No v
  transposes.
- Scores keep the [k-part, q-free] orientation; exp runs on the scalar
  engine over groups of up to 3 PSUM banks into SBUF bf16 tiles (eb).
  Above-diagonal q columns are trimmed from the score matmuls (exp of
  the resulting zeroed columns is never consumed).
- att@v is reoriented to out [q-part, 65]: lhsT = eb column block,
  rhs = v_aug [k, 65] (v plus a ones column). The 65th output column
  is the softmax denominator, per-partition, so normalization is a
  reciprocal + tensor_scalar multiply - no partition broadcast.
- The normalized attention output [q, d] is transposed back (PE, bf16)
  and packed into aoT01 [128, T] / aoT2 [64, T] so the output
  projection contracts 128+64 at a time.
- PSUM: a pool of three 2-bank "slot" tiles serves scores (pairs of
  k-tiles, exp'd as one [128,<=1024] activation), qkv slots, transposes
  and proj outputs; 2 banks hold att accumulators. Pool-tile rotation
  gives exact per-slot WAR/RAW deps - slicing one big PSUM tile instead
  serializes everything (PSUM dep tracking is coarse).
- Scheduling: exp on the Act engine (~220us busy) and matmul rows on
  PE (~202us) are the co-bottlenecks. The tile scheduler lowers each
  exp's dependencies to "all PE work emitted so far", so each score
  pair is immediately followed by its exp, and all other PE work
  (qkv, att@v, transposes, proj) is queued as (cost, closure) items
  popped between rounds against a budget fitted to each exp's
  duration. qkv work is a priority queue (it releases x-tile buffers
  and feeds the next stream's scores). DVE does the PSUM->SBUF
  copies/normalize (GPSIMD cannot read PSUM on real HW); Pool does
  the causal masks (affine_select on eb) and some weight-load DMAs.
  Head h's att@v+normalize work drains during head h+1's score/exp
  stream; eb tiles (bf16 exp outputs) are triple-buffered in SBUF.
"""

import sys

for _p in ("/opt/trn_rl_repo",):
    if _p not in sys.path:
        sys.path.insert(0, _p)

from collections import deque
from contextlib import ExitStack

import numpy as np

import concourse.bass as bass  # noqa: F401
import concourse.mybir as mybir
import concourse.tile as tile
from concourse import bacc
from concourse.bass_utils import run_bass_kernel_spmd
from concourse.masks import make_identity

f32 = mybir.dt.float32
bf16 = mybir.dt.bfloat16
AF = mybir.ActivationFunctionType

C = 768
D = 64
N_HEAD = 12
HPC = 3  # heads per core
N_CORES = 8


def build_nc(T):
    NT = T // 512  # q tiles
    KT = T // 128  # k tiles
    CK = C // 128  # contraction chunks

    nc = bacc.Bacc("TRN2", target_bir_lowering=False, debug=False,
                   num_devices=N_CORES)
    xt_d = nc.dram_tensor("xt", [C, T], bf16, kind="ExternalInput").ap()
    wqv_d = nc.dram_tensor("wqv", [C, 576], bf16, kind="ExternalInput").ap()
    bv_d = nc.dram_tensor("bv", [1, 192], bf16, kind="ExternalInput").ap()
    bqk_d = nc.dram_tensor("bqk", [128, 3], f32, kind="ExternalInput").ap()
    wp01_d = nc.dram_tensor("wp01", [128, C], bf16, kind="ExternalInput").ap()
    wp2_d = nc.dram_tensor("wp2", [64, C], bf16, kind="ExternalInput").ap()
    y_d = nc.dram_tensor("y", [T, C], f32, kind="ExternalOutput").ap()

    with tile.TileContext(nc) as tc, ExitStack() as ctx:
        sb = ctx.enter_context(tc.tile_pool(name="sb", bufs=1))

        # ---- persistent SBUF ----
        wqv_sb = [sb.tile([128, 576], bf16, tag=f"wqv{c}", name=f"wqv{c}")
                  for c in range(CK)]
        bv_sb = sb.tile([1, 192], bf16, tag="bv")
        bqk_sb = sb.tile([128, 3], f32, tag="bqk")
        wp01_sb = sb.tile([128, C], bf16, tag="wp01")
        wp2_sb = sb.tile([64, C], bf16, tag="wp2")
        ident = sb.tile([128, 128], bf16, tag="ident")
        ones1 = sb.tile([1, 128], bf16, tag="ones1")
        qT_AB = sb.tile([128, T], bf16, tag="qAB")
        kT_AB = sb.tile([128, T], bf16, tag="kAB")
        qT_C = sb.tile([64, T], bf16, tag="qC")
        kT_C = sb.tile([128, T], bf16, tag="kC")
        v_aug = sb.tile([128, KT * 195], bf16, tag="vaug")
        aoT_pool = ctx.enter_context(tc.tile_pool(name="aotp", bufs=3))

        # ---- pools ----
        xt_pool = ctx.enter_context(tc.tile_pool(name="xtp", bufs=13))
        y_pool = ctx.enter_context(tc.tile_pool(name="yp", bufs=6))
        eb_pool = ctx.enter_context(tc.tile_pool(name="ebp", bufs=3))
        attn_pool = ctx.enter_context(tc.tile_pool(name="attnp", bufs=4))
        rcp_pool = ctx.enter_context(tc.tile_pool(name="rcpp", bufs=6))
        ring_pool = ctx.enter_context(
            tc.tile_pool(name="ringp", bufs=2, space="PSUM"))
        att_pool = ctx.enter_context(
            tc.tile_pool(name="attp", bufs=2, space="PSUM"))

        # startup: x tile DMAs for j=0 first, then weights in use order
        xt0 = [xt_pool.tile([128, 512], bf16, tag="xt", name="xt0")
               for _ in range(CK)]
        for c in range(CK):
            nc.sync.dma_start(xt0[c][:], xt_d[c * 128:(c + 1) * 128, 0:512])
            # weight chunks split across the Act and Pool DMA queues so the
            # first qkv chains start ~6us earlier
            if c < 3:
                nc.scalar.dma_start(out=wqv_sb[c][:],
                                    in_=wqv_d[c * 128:(c + 1) * 128, :])
            else:
                nc.gpsimd.dma_start(out=wqv_sb[c][:],
                                    in_=wqv_d[c * 128:(c + 1) * 128, :])
            if c == 2:
                # bqk feeds the first qkv copy-out (~5us in): don't queue it
                # behind all six x-tile DMAs
                nc.sync.dma_start(bqk_sb[:], bqk_d)
        nc.sync.dma_start(bv_sb[:], bv_d)
        nc.sync.dma_start(wp01_sb[:], wp01_d)
        nc.sync.dma_start(wp2_sb[:], wp2_d)
        make_identity(nc, ident[:])
        nc.vector.memset(ones1[:], 1.0)
        va_r = v_aug[:].rearrange("p (k c) -> p k c", c=195)
        for h in range(HPC):
            nc.gpsimd.memset(va_r[:, :, 65 * h + 64:65 * h + 65], 1.0)

        def next_slot():
            """Claim a 2-bank PSUM slot; pool rotation provides exact
            per-slot WAR/RAW dependencies."""
            return ring_pool.tile([128, 1536], f32, tag="ring", name="slot")

        # ------------- deferred-work queue -------------
        # items are (pe_cost_ns, closure); popped between score groups
        # against a per-round budget so the next score group is never far
        # behind in the in-order PE queue.
        import os
        sched_dbg = os.environ.get("KDBG_SCHED") == "1"
        dq = deque()
        uq = deque()  # urgent: qkv work (releases xt tiles, feeds scores)
        dq_cost = [0.0]
        gen_state = {"cur": 0}

        def push(cost, fn):
            dq.append((cost, fn, gen_state["cur"]))
            dq_cost[0] += cost

        def push_urgent(cost, fn):
            uq.append((cost, fn))
            dq_cost[0] += cost

        def pop_budget(budget):
            spent = 0.0
            while uq and spent < max(budget, 900.0):
                cost, fn = uq.popleft()
                dq_cost[0] -= cost
                fn()
                spent += cost
            while dq and spent < budget:
                cost, fn, g = dq.popleft()
                dq_cost[0] -= cost
                fn()
                spent += cost

        # ------------- emission helpers -------------
        def emit_qkv(j, xt_pre=None):
            """Queue q/k/v production for q-tile j."""
            jsl = bass.ts(j, 512)
            xt_t = list(xt_pre) if xt_pre is not None else [None] * CK

            def clo_dma():
                for c in range(CK):
                    xt_t[c] = xt_pool.tile([128, 512], bf16, tag="xt",
                                           name="xt")
                    nc.sync.dma_start(
                        xt_t[c][:],
                        xt_d[c * 128:(c + 1) * 128, j * 512:(j + 1) * 512])

            def mk_qk(s):
                state = {}

                def clo_a():
                    state["sl"] = next_slot()
                    out = state["sl"][:, 0:512]
                    for c in range(3):
                        nc.tensor.matmul(out,
                                         wqv_sb[c][:, s * 128:(s + 1) * 128],
                                         xt_t[c][:], start=(c == 0),
                                         stop=False)

                def clo_b():
                    out = state["sl"][:, 0:512]
                    for c in range(3, CK):
                        nc.tensor.matmul(out,
                                         wqv_sb[c][:, s * 128:(s + 1) * 128],
                                         xt_t[c][:], start=False,
                                         stop=(c == CK - 1))
                    if s == 0:
                        nc.vector.tensor_scalar_add(qT_AB[:, jsl], out,
                                                    bqk_sb[:, 0:1])
                    elif s == 1:
                        nc.vector.tensor_scalar_add(kT_AB[:, jsl], out,
                                                    bqk_sb[:, 1:2])
                    else:
                        nc.vector.tensor_scalar_add(qT_C[:, jsl],
                                                    out[0:64, :],
                                                    bqk_sb[0:64, 2:3])
                        nc.vector.tensor_scalar_add(kT_C[64:128, jsl],
                                                    out[64:128, :],
                                                    bqk_sb[64:128, 2:3])
                        # k2 must also live on partitions 0:64 (score lhsT)
                        nc.sync.dma_start(kT_C[0:64, jsl], kT_C[64:128, jsl])
                return clo_a, clo_b

            def mk_v(kt):
                def clo():
                    ki = 4 * j + kt
                    out = next_slot()[:, 0:192]
                    for c in range(CK):
                        nc.tensor.matmul(
                            out, xt_t[c][:, kt * 128:(kt + 1) * 128],
                            wqv_sb[c][:, 384:576], start=(c == 0), stop=False)
                    nc.tensor.matmul(out, ones1[:], bv_sb[:],
                                     start=False, stop=True)
                    dst = va_r[:, ki:ki + 1, :].rearrange(
                        "p k (h c) -> p (k h) c", c=65)[:, :, 0:64]
                    src = out.rearrange("p (h c) -> p h c", c=64)
                    nc.vector.tensor_copy(dst, src)
                return clo

            qk_part = []
            for s in range(3):
                ca, cb = mk_qk(s)
                qk_part.append((640, ca))
                qk_part.append((640, cb))
            v_part = [(600, mk_v(kt)) for kt in range(4)]
            dma_part = [] if xt_pre is not None else [(50, clo_dma)]
            return dma_part, qk_part, v_part

        def head_qk(h):
            if h == 0:
                return kT_AB[0:64, :], qT_AB[0:64, :]
            if h == 1:
                return kT_AB[64:128, :], qT_AB[64:128, :]
            return kT_C[0:64, :], qT_C[:]

        def emit_scores(j, h, grp):
            kt_src, qt_src = head_qk(h)
            slot = next_slot()
            for idx, ki in enumerate(grp):
                r = ki - 4 * j
                t = 128 * r if r > 0 else 0  # diagonal q-trim
                nc.tensor.matmul(
                    slot[:, idx * 512 + t:(idx + 1) * 512],
                    kt_src[:, bass.ts(ki, 128)],
                    qt_src[:, j * 512 + t:(j + 1) * 512],
                    start=True, stop=True)
            return slot

        def emit_exp(j, eb_t, grp, slot):
            ncols = 512 * len(grp)
            r0 = grp[0] - 4 * j
            t0 = 128 * r0 if r0 > 0 else 0  # columns before t0 are never read
            nc.scalar.activation(
                eb_t[:, grp[0] * 512 + t0:grp[0] * 512 + ncols],
                slot[:, t0:ncols], AF.Exp, scale=0.125)
            for ki in grp:
                r = ki - 4 * j
                if r >= 0:
                    # causal mask inside the diagonal 128x128 block
                    blk = eb_t[:, ki * 512 + 128 * r:ki * 512 + 128 * r + 128]
                    nc.gpsimd.affine_select(
                        blk, blk, pattern=[[1, 128]],
                        compare_op=mybir.AluOpType.is_ge, fill=0.0,
                        base=0, channel_multiplier=-1)

        pending = [None]  # (j, eb_t, grp, s0) carried across streams

        def emit_stream(j, h, eb_t, tail=None, late=None):
            """Score+exp stream for (j, h): scores run one group ahead of
            exp (across stream boundaries too); deferred work fills the
            remaining PE time each round. `late` = (round_idx, [fns]) emitted
            right after that round (last-stream tail shortening)."""
            nk = 4 * j + 4
            # groups of 2 banks: 3 score groups in flight in the 6-bank
            # ring, deep enough to hide the exp write-ack latency
            groups = [list(range(g, min(g + 3, nk))) for g in range(0, nk, 3)]
            if sched_dbg:
                oldest = dq[0][2] if dq else -1
                print(f"stream j={j} h={h} gen={gen_state['cur']} "
                      f"qlen={len(dq)} qcost={dq_cost[0]:.0f} oldest_gen={oldest}")
            for gi, grp in enumerate(groups):
                s0 = emit_scores(j, h, grp)
                # exp immediately after its scores: the scheduler lowers the
                # exp's deps to "all PE work emitted so far", so nothing else
                # may sit between the scores and their exp
                emit_exp(j, eb_t, grp, s0)
                rounds_left = len(groups) - gi
                budget = max(500.0, min(900.0, dq_cost[0] / rounds_left))
                pop_budget(budget)
            gen_state["cur"] += 1

        def emit_attv(j, h, eb_t, sink=None):
            """Queue att@v chunks + normalize (+ tails at h==2)."""
            sink = sink if sink is not None else push
            jsl_base = j * 512

            def mk_chunk(qc, k0, k1, att_box, first, last):
                def clo():
                    if first:
                        att_box[0] = att_pool.tile([128, 65], f32, tag="att",
                                                   name="att")
                    att = att_box[0]
                    nkq = 4 * j + qc + 1
                    for ki in range(k0, k1):
                        nc.tensor.matmul(
                            att[:],
                            eb_t[:, ki * 512 + qc * 128:ki * 512 + qc * 128 + 128],
                            v_aug[:, ki * 195 + 65 * h:ki * 195 + 65 * h + 65],
                            start=(ki == 0), stop=(ki == nkq - 1))
                    if last:
                        rcp = rcp_pool.tile([128, 1], f32, tag="rcp",
                                            name="rcp")
                        nc.vector.reciprocal_approx_fast(out=rcp[:],
                                                         in_=att[:, 64:65])
                        if h == 0:
                            attn_t[qc] = attn_pool.tile([128, 192], bf16,
                                                        tag=f"attn{qc}",
                                                        name="attn")
                        nc.vector.tensor_scalar_mul(
                            attn_t[qc][:, h * 64:(h + 1) * 64], att[:, 0:64],
                            rcp[:])
                return clo

            for qc in range(4):
                nkq = 4 * j + qc + 1
                att_box = [None]
                # split long accumulations into <=16-matmul pieces
                k0 = 0
                while k0 < nkq:
                    k1 = min(k0 + 16, nkq)
                    sink(27 * (k1 - k0) + (100 if k1 == nkq else 0),
                         mk_chunk(qc, k0, k1, att_box, k0 == 0, k1 == nkq))
                    k0 = k1

            if h == 2:
                ao_box = [None, None]
                for qc in range(4):
                    sink(450, mk_transpose(j, qc, ao_box))
                for qc in range(4):
                    for ns in range(2):
                        sink(550, mk_proj(j, qc, ns, ao_box))

        def mk_transpose(j, qc, ao_box):
            def clo():
                if qc == 0:
                    ao_box[0] = aoT_pool.tile([128, 512], bf16, tag="ao01",
                                              name="ao01")
                    ao_box[1] = aoT_pool.tile([64, 512], bf16, tag="ao2",
                                              name="ao2")
                tps = next_slot().bitcast(bf16)
                tp1 = tps[:, 0:1024]
                tp2 = tps[:, 1024:2048]
                nc.tensor.transpose(tp1[:, 0:128], attn_t[qc][:, 0:128],
                                    ident[:])
                nc.tensor.transpose(tp2[0:64, 0:128], attn_t[qc][:, 128:192],
                                    ident[:])
                csl = slice(qc * 128, qc * 128 + 128)
                nc.vector.tensor_copy(ao_box[0][:, csl], tp1[:, 0:128])
                nc.vector.tensor_copy(ao_box[1][:, csl], tp2[0:64, 0:128])
            return clo

        def mk_proj(j, qc, ns, ao_box):
            def clo():
                msl = slice(j * 512 + qc * 128, j * 512 + qc * 128 + 128)
                csl = slice(qc * 128, qc * 128 + 128)
                py = next_slot()[:, 0:384]
                nc.tensor.matmul(py, ao_box[0][:, csl],
                                 wp01_sb[:, ns * 384:(ns + 1) * 384],
                                 start=True, stop=False)
                nc.tensor.matmul(py, ao_box[1][:, csl],
                                 wp2_sb[:, ns * 384:(ns + 1) * 384],
                                 start=False, stop=True)
                y_sb = y_pool.tile([128, 384], f32, tag="y", name="ysb")
                nc.vector.tensor_copy(y_sb[:], py)
                nc.sync.dma_start(y_d[msl, ns * 384:(ns + 1) * 384], y_sb[:])
            return clo

        # ------------- main pipeline -------------
        attn_t = [None] * 4  # per-q-chunk staging tiles (rebound per j)
        _, qk0, v0 = emit_qkv(0, xt_pre=xt0)
        for _, fn in qk0[:4]:
            fn()  # bootstrap: q01/k01 chains only
        for item in qk0[4:] + v0:
            push_urgent(*item)  # slot2 + v hide inside stream (0,0)
        parts = {}
        if NT > 1:
            parts[1] = emit_qkv(1)
            for item in parts[1][0]:
                item[1]()  # xt prefetch for j=1 up front
        for j in range(NT):
            for h in range(HPC):
                eb_t = eb_pool.tile([128, (NT * 4) * 512], bf16, tag="eb",
                                    name="eb")
                if h == 0 and j + 1 < NT:
                    for item in parts[j + 1][1]:
                        push_urgent(*item)
                elif h == 1 and j + 1 < NT:
                    for item in parts[j + 1][2]:
                        push_urgent(*item)
                elif h == 2 and j + 2 < NT:
                    parts[j + 2] = emit_qkv(j + 2)
                    for item in parts[j + 2][0]:
                        push_urgent(*item)  # xt prefetch two tiles ahead
                last = (j == NT - 1 and h == 2)
                if last:
                    # chunks 0/1 (and their tails) only need exps through
                    # round nk//2 - 2; emit them mid-stream so only chunks
                    # 2/3 trail the final exp
                    items = []
                    emit_attv(j, h, eb_t,
                              sink=lambda c, f: items.append(f))
                    early = items[0:4] + items[8:10] + items[12:16]
                    rest = [f for f in items if f not in early]
                    nrounds = (4 * j + 4 + 1) // 2
                    emit_stream(j, h, eb_t, tail=lambda: [f() for f in rest],
                                late=(nrounds - 2, early))
                else:
                    emit_stream(j, h, eb_t)
                    emit_attv(j, h, eb_t)
        pop_budget(float("inf"))

    nc.compile()
    return nc


_NC_CACHE = {}


def _get_nc(T):
    if T not in _NC_CACHE:
        _NC_CACHE[T] = build_nc(T)
    return _NC_CACHE[T]


def make_core_inputs(x, W_attn, b_attn, W_proj):
    """Host-side prep: per-core input dicts (see module docstring)."""
    import ml_dtypes
    B, T, _ = x.shape
    xts = [np.ascontiguousarray(x[b].T).astype(ml_dtypes.bfloat16)
           for b in range(B)]
    in_maps = []
    for core in range(N_CORES):
        b = core // (N_CORES // B)
        h0 = HPC * (core % (N_CORES // B))
        c01 = slice(h0 * D, (h0 + 2) * D)      # first two heads
        c2 = slice((h0 + 2) * D, (h0 + 3) * D)  # third head
        # reference splits qkv as (k, q, v): k cols 0:C, q cols C:2C, v 2C:3C
        q01 = W_attn[:, C:2 * C][:, c01]
        k01 = W_attn[:, 0:C][:, c01]
        q2 = W_attn[:, C:2 * C][:, c2]
        k2 = W_attn[:, 0:C][:, c2]
        wv = W_attn[:, 2 * C:3 * C][:, h0 * D:(h0 + 3) * D]
        wqv = np.concatenate([q01, k01, q2, k2, wv], axis=1)
        bqk = np.zeros((128, 3), np.float32)
        bqk[:, 0] = b_attn[C:2 * C][c01]
        bqk[:, 1] = b_attn[0:C][c01]
        bqk[0:64, 2] = b_attn[C:2 * C][c2]
        bqk[64:128, 2] = b_attn[0:C][c2]
        bv = b_attn[2 * C:3 * C][h0 * D:(h0 + 3) * D].reshape(1, 192)
        wp01 = W_proj[h0 * D:(h0 + 2) * D, :]
        wp2 = W_proj[(h0 + 2) * D:(h0 + 3) * D, :]
        in_maps.append({
            "xt": xts[b],
            "wqv": np.ascontiguousarray(wqv).astype(ml_dtypes.bfloat16),
            "bv": np.ascontiguousarray(bv).astype(ml_dtypes.bfloat16),
            "bqk": bqk,
            "wp01": np.ascontiguousarray(wp01).astype(ml_dtypes.bfloat16),
            "wp2": np.ascontiguousarray(wp2).astype(ml_dtypes.bfloat16),
        })
    return in_maps


def kernel(x, W_attn, b_attn, W_proj, b_proj):
    x = np.asarray(x, dtype=np.float32)
    W_attn = np.asarray(W_attn, dtype=np.float32)
    b_attn = np.asarray(b_attn, dtype=np.float32)
    W_proj = np.asarray(W_proj, dtype=np.float32)
    b_proj = np.asarray(b_proj, dtype=np.float32)
    B, T, _ = x.shape

    nc = _get_nc(T)
    in_maps = make_core_inputs(x, W_attn, b_attn, W_proj)
    res = None
    for attempt in range(3):
        try:
            res = run_bass_kernel_spmd(nc, in_maps, list(range(N_CORES)))
            break
        except Exception:
            # transient NRT_EXEC_UNIT_UNRECOVERABLE has been observed once
            # after a prior crashed process; a retry succeeds
            if attempt == 2:
                raise
    global LAST_RUN
    LAST_RUN = res

    gpb = N_CORES // B
    out = np.empty((B, T, C), np.float32)
    for b in range(B):
        acc = res.results[b * gpb]["y"].astype(np.float32)
        for g in range(1, gpb):
            acc = acc + res.results[b * gpb + g]["y"]
        out[b] = acc + b_proj[None, :]
    return out


# revision 57
# speedup vs baseline: 1.4033x; 1.0048x over previous
"""Causal self-attention (B=2, T=4096, C=768, H=12) on 8 TRN2 NeuronCores.

Sharding: batch x head-group. Core c handles batch b=c//4 and heads
h0..h0+2 where h0 = 3*(c%4). Each core computes the qkv projection for
its 3 heads, full causal attention, and a partial output projection; the
host sums the 4 partials per batch and adds the projection bias.

v2 design (all matmul inputs bf16):
- q/k are produced transposed ([d, T]); v is produced directly in
  natural layout [k, d] per 128-k-tile (lhsT = x^T tile, rhs = Wv
  chunk), with the v bias folded in via a ones-row matmul. No v
  transposes.
- Scores keep the [k-part, q-free] orientation; exp runs on the scalar
  engine over groups of up to 3 PSUM banks into SBUF bf16 tiles (eb).
  Above-diagonal q columns are trimmed from the score matmuls (exp of
  the resulting zeroed columns is never consumed).
- att@v is reoriented to out [q-part, 65]: lhsT = eb column block,
  rhs = v_aug [k, 65] (v plus a ones column). The 65th output column
  is the softmax denominator, per-partition, so normalization is a
  reciprocal + tensor_scalar multiply - no partition broadcast.
- The normalized attention output [q, d] is transposed back (PE, bf16)
  and packed into aoT01 [128, T] / aoT2 [64, T] so the output
  projection contracts 128+64 at a time.
- PSUM: a pool of three 2-bank "slot" tiles serves scores (pairs of
  k-tiles, exp'd as one [128,<=1024] activation), qkv slots, transposes
  and proj outputs; 2 banks hold att accumulators. Pool-tile rotation
  gives exact per-slot WAR/RAW deps - slicing one big PSUM tile instead
  serializes everything (PSUM dep tracking is coarse).
- Scheduling: exp on the Act engine (~220us busy) and matmul rows on
  PE (~202us) are the co-bottlenecks. The tile scheduler lowers each
  exp's dependencies to "all PE work emitted so far", so each score
  pair is immediately followed by its exp, and all other PE work
  (qkv, att@v, transposes, proj) is queued as (cost, closure) items
  popped between rounds against a budget fitted to each exp's
  duration. qkv work is a priority queue (it releases x-tile buffers
  and feeds the next stream's scores). DVE does the PSUM->SBUF
  copies/normalize (GPSIMD cannot read PSUM on real HW); Pool does
  the causal masks (affine_select on eb) and some weight-load DMAs.
  Head h's att@v+normalize work drains during head h+1's score/exp
  stream; eb tiles (bf16 exp outputs) are triple-buffered in SBUF.
"""

import sys

for _p in ("/opt/trn_rl_repo",):
    if _p not in sys.path:
        sys.path.insert(0, _p)

from collections import deque
from contextlib import ExitStack

import numpy as np

import concourse.bass as bass  # noqa: F401
import concourse.mybir as mybir
import concourse.tile as tile
from concourse import bacc
from concourse.bass_utils import run_bass_kernel_spmd
from concourse.masks import make_identity

f32 = mybir.dt.float32
bf16 = mybir.dt.bfloat16
AF = mybir.ActivationFunctionType

C = 768
D = 64
N_HEAD = 12
HPC = 3  # heads per core
N_CORES = 8


def build_nc(T):
    NT = T // 512  # q tiles
    KT = T // 128  # k tiles
    CK = C // 128  # contraction chunks

    nc = bacc.Bacc("TRN2", target_bir_lowering=False, debug=False,
                   num_devices=N_CORES)
    xt_d = nc.dram_tensor("xt", [C, T], bf16, kind="ExternalInput").ap()
    wqv_d = nc.dram_tensor("wqv", [C, 576], bf16, kind="ExternalInput").ap()
    bv_d = nc.dram_tensor("bv", [1, 192], bf16, kind="ExternalInput").ap()
    bqk_d = nc.dram_tensor("bqk", [128, 3], f32, kind="ExternalInput").ap()
    wp01_d = nc.dram_tensor("wp01", [128, C], bf16, kind="ExternalInput").ap()
    wp2_d = nc.dram_tensor("wp2", [64, C], bf16, kind="ExternalInput").ap()
    y_d = nc.dram_tensor("y", [T, C], f32, kind="ExternalOutput").ap()

    with tile.TileContext(nc) as tc, ExitStack() as ctx:
        sb = ctx.enter_context(tc.tile_pool(name="sb", bufs=1))

        # ---- persistent SBUF ----
        wqv_sb = [sb.tile([128, 576], bf16, tag=f"wqv{c}", name=f"wqv{c}")
                  for c in range(CK)]
        bv_sb = sb.tile([1, 192], bf16, tag="bv")
        bqk_sb = sb.tile([128, 3], f32, tag="bqk")
        wp01_sb = sb.tile([128, C], bf16, tag="wp01")
        wp2_sb = sb.tile([64, C], bf16, tag="wp2")
        ident = sb.tile([128, 128], bf16, tag="ident")
        ones1 = sb.tile([1, 128], bf16, tag="ones1")
        qT_AB = sb.tile([128, T], bf16, tag="qAB")
        kT_AB = sb.tile([128, T], bf16, tag="kAB")
        qT_C = sb.tile([64, T], bf16, tag="qC")
        kT_C = sb.tile([128, T], bf16, tag="kC")
        v_aug = sb.tile([128, KT * 195], bf16, tag="vaug")
        aoT_pool = ctx.enter_context(tc.tile_pool(name="aotp", bufs=3))

        # ---- pools ----
        xt_pool = ctx.enter_context(tc.tile_pool(name="xtp", bufs=13))
        y_pool = ctx.enter_context(tc.tile_pool(name="yp", bufs=6))
        eb_pool = ctx.enter_context(tc.tile_pool(name="ebp", bufs=3))
        attn_pool = ctx.enter_context(tc.tile_pool(name="attnp", bufs=4))
        rcp_pool = ctx.enter_context(tc.tile_pool(name="rcpp", bufs=6))
        ring_pool = ctx.enter_context(
            tc.tile_pool(name="ringp", bufs=2, space="PSUM"))
        att_pool = ctx.enter_context(
            tc.tile_pool(name="attp", bufs=2, space="PSUM"))

        # startup: x tile DMAs for j=0 first, then weights in use order
        xt0 = [xt_pool.tile([128, 512], bf16, tag="xt", name="xt0")
               for _ in range(CK)]
        for c in range(CK):
            nc.sync.dma_start(xt0[c][:], xt_d[c * 128:(c + 1) * 128, 0:512])
            # weight chunks split across the Act and Pool DMA queues so the
            # first qkv chains start ~6us earlier
            if c < 3:
                nc.scalar.dma_start(out=wqv_sb[c][:],
                                    in_=wqv_d[c * 128:(c + 1) * 128, :])
            else:
                nc.gpsimd.dma_start(out=wqv_sb[c][:],
                                    in_=wqv_d[c * 128:(c + 1) * 128, :])
            if c == 2:
                # bqk feeds the first qkv copy-out (~5us in): don't queue it
                # behind all six x-tile DMAs
                nc.sync.dma_start(bqk_sb[:], bqk_d)
        nc.sync.dma_start(bv_sb[:], bv_d)
        nc.sync.dma_start(wp01_sb[:], wp01_d)
        nc.sync.dma_start(wp2_sb[:], wp2_d)
        make_identity(nc, ident[:])
        nc.vector.memset(ones1[:], 1.0)
        va_r = v_aug[:].rearrange("p (k c) -> p k c", c=195)
        for h in range(HPC):
            nc.gpsimd.memset(va_r[:, :, 65 * h + 64:65 * h + 65], 1.0)

        def next_slot():
            """Claim a 2-bank PSUM slot; pool rotation provides exact
            per-slot WAR/RAW dependencies."""
            return ring_pool.tile([128, 1536], f32, tag="ring", name="slot")

        # ------------- deferred-work queue -------------
        # items are (pe_cost_ns, closure); popped between score groups
        # against a per-round budget so the next score group is never far
        # behind in the in-order PE queue.
        import os
        sched_dbg = os.environ.get("KDBG_SCHED") == "1"
        dq = deque()
        uq = deque()  # urgent: qkv work (releases xt tiles, feeds scores)
        dq_cost = [0.0]
        gen_state = {"cur": 0}

        def push(cost, fn):
            dq.append((cost, fn, gen_state["cur"]))
            dq_cost[0] += cost

        def push_urgent(cost, fn):
            uq.append((cost, fn))
            dq_cost[0] += cost

        def pop_budget(budget):
            spent = 0.0
            while uq and spent < max(budget, 900.0):
                cost, fn = uq.popleft()
                dq_cost[0] -= cost
                fn()
                spent += cost
            while dq and spent < budget:
                cost, fn, g = dq.popleft()
                dq_cost[0] -= cost
                fn()
                spent += cost

        # ------------- emission helpers -------------
        def emit_qkv(j, xt_pre=None):
            """Queue q/k/v production for q-tile j."""
            jsl = bass.ts(j, 512)
            xt_t = list(xt_pre) if xt_pre is not None else [None] * CK

            def clo_dma():
                for c in range(CK):
                    xt_t[c] = xt_pool.tile([128, 512], bf16, tag="xt",
                                           name="xt")
                    nc.sync.dma_start(
                        xt_t[c][:],
                        xt_d[c * 128:(c + 1) * 128, j * 512:(j + 1) * 512])

            def mk_qk(s):
                state = {}

                def clo_a():
                    state["sl"] = next_slot()
                    out = state["sl"][:, 0:512]
                    for c in range(3):
                        nc.tensor.matmul(out,
                                         wqv_sb[c][:, s * 128:(s + 1) * 128],
                                         xt_t[c][:], start=(c == 0),
                                         stop=False)

                def clo_b():
                    out = state["sl"][:, 0:512]
                    for c in range(3, CK):
                        nc.tensor.matmul(out,
                                         wqv_sb[c][:, s * 128:(s + 1) * 128],
                                         xt_t[c][:], start=False,
                                         stop=(c == CK - 1))
                    if s == 0:
                        nc.vector.tensor_scalar_add(qT_AB[:, jsl], out,
                                                    bqk_sb[:, 0:1])
                    elif s == 1:
                        nc.vector.tensor_scalar_add(kT_AB[:, jsl], out,
                                                    bqk_sb[:, 1:2])
                    else:
                        nc.vector.tensor_scalar_add(qT_C[:, jsl],
                                                    out[0:64, :],
                                                    bqk_sb[0:64, 2:3])
                        nc.vector.tensor_scalar_add(kT_C[64:128, jsl],
                                                    out[64:128, :],
                                                    bqk_sb[64:128, 2:3])
                        # k2 must also live on partitions 0:64 (score lhsT)
                        nc.sync.dma_start(kT_C[0:64, jsl], kT_C[64:128, jsl])
                return clo_a, clo_b

            def mk_v(kt):
                # two k-tiles share one 2-bank slot (one tile per bank, so
                # start=True bank-zeroing stays within each tile's bank)
                def clo():
                    slot = next_slot()
                    for p in range(2):
                        out = slot[:, 512 * p:512 * p + 192]
                        for c in range(CK):
                            nc.tensor.matmul(
                                out,
                                xt_t[c][:, (kt + p) * 128:(kt + p + 1) * 128],
                                wqv_sb[c][:, 384:576], start=(c == 0),
                                stop=False)
                        nc.tensor.matmul(out, ones1[:], bv_sb[:],
                                         start=False, stop=True)
                    for p in range(2):
                        ki = 4 * j + kt + p
                        dst = va_r[:, ki:ki + 1, :].rearrange(
                            "p k (h c) -> p (k h) c", c=65)[:, :, 0:64]
                        src = slot[:, 512 * p:512 * p + 192].rearrange(
                            "p (h c) -> p h c", c=64)
                        nc.vector.tensor_copy(dst, src)
                return clo

            qk_part = []
            for s in range(3):
                ca, cb = mk_qk(s)
                qk_part.append((640, ca))
                qk_part.append((640, cb))
            v_part = [(1250, mk_v(kt)) for kt in (0, 2)]
            dma_part = [] if xt_pre is not None else [(50, clo_dma)]
            return dma_part, qk_part, v_part

        def head_qk(h):
            if h == 0:
                return kT_AB[0:64, :], qT_AB[0:64, :]
            if h == 1:
                return kT_AB[64:128, :], qT_AB[64:128, :]
            return kT_C[0:64, :], qT_C[:]

        def emit_scores(j, h, grp):
            kt_src, qt_src = head_qk(h)
            slot = next_slot()
            for idx, ki in enumerate(grp):
                r = ki - 4 * j
                t = 128 * r if r > 0 else 0  # diagonal q-trim
                nc.tensor.matmul(
                    slot[:, idx * 512 + t:(idx + 1) * 512],
                    kt_src[:, bass.ts(ki, 128)],
                    qt_src[:, j * 512 + t:(j + 1) * 512],
                    start=True, stop=True)
            return slot

        def emit_exp(j, eb_t, grp, slot):
            ncols = 512 * len(grp)
            r0 = grp[0] - 4 * j
            t0 = 128 * r0 if r0 > 0 else 0  # columns before t0 are never read
            nc.scalar.activation(
                eb_t[:, grp[0] * 512 + t0:grp[0] * 512 + ncols],
                slot[:, t0:ncols], AF.Exp, scale=0.125)
            for ki in grp:
                r = ki - 4 * j
                if r >= 0:
                    # causal mask inside the diagonal 128x128 block
                    blk = eb_t[:, ki * 512 + 128 * r:ki * 512 + 128 * r + 128]
                    nc.gpsimd.affine_select(
                        blk, blk, pattern=[[1, 128]],
                        compare_op=mybir.AluOpType.is_ge, fill=0.0,
                        base=0, channel_multiplier=-1)

        pending = [None]  # (j, eb_t, grp, s0) carried across streams

        def emit_stream(j, h, eb_t, tail=None, late=None):
            """Score+exp stream for (j, h): scores run one group ahead of
            exp (across stream boundaries too); deferred work fills the
            remaining PE time each round. `late` = (round_idx, [fns]) emitted
            right after that round (last-stream tail shortening)."""
            nk = 4 * j + 4
            # groups of 2 banks: 3 score groups in flight in the 6-bank
            # ring, deep enough to hide the exp write-ack latency
            groups = [list(range(g, min(g + 3, nk))) for g in range(0, nk, 3)]
            if sched_dbg:
                oldest = dq[0][2] if dq else -1
                print(f"stream j={j} h={h} gen={gen_state['cur']} "
                      f"qlen={len(dq)} qcost={dq_cost[0]:.0f} oldest_gen={oldest}")
            for gi, grp in enumerate(groups):
                s0 = emit_scores(j, h, grp)
                # exp immediately after its scores: the scheduler lowers the
                # exp's deps to "all PE work emitted so far", so nothing else
                # may sit between the scores and their exp
                emit_exp(j, eb_t, grp, s0)
                rounds_left = len(groups) - gi
                budget = max(500.0, min(900.0, dq_cost[0] / rounds_left))
                pop_budget(budget)
            gen_state["cur"] += 1

        def emit_attv(j, h, eb_t, sink=None):
            """Queue att@v chunks + normalize (+ tails at h==2)."""
            sink = sink if sink is not None else push
            jsl_base = j * 512

            def mk_chunk(qc, k0, k1, att_box, first, last):
                def clo():
                    if first:
                        att_box[0] = att_pool.tile([128, 65], f32, tag="att",
                                                   name="att")
                    att = att_box[0]
                    nkq = 4 * j + qc + 1
                    for ki in range(k0, k1):
                        nc.tensor.matmul(
                            att[:],
                            eb_t[:, ki * 512 + qc * 128:ki * 512 + qc * 128 + 128],
                            v_aug[:, ki * 195 + 65 * h:ki * 195 + 65 * h + 65],
                            start=(ki == 0), stop=(ki == nkq - 1))
                    if last:
                        rcp = rcp_pool.tile([128, 1], f32, tag="rcp",
                                            name="rcp")
                        nc.vector.reciprocal_approx_fast(out=rcp[:],
                                                         in_=att[:, 64:65])
                        if h == 0:
                            attn_t[qc] = attn_pool.tile([128, 192], bf16,
                                                        tag=f"attn{qc}",
                                                        name="attn")
                        nc.vector.tensor_scalar_mul(
                            attn_t[qc][:, h * 64:(h + 1) * 64], att[:, 0:64],
                            rcp[:])
                return clo

            for qc in range(4):
                nkq = 4 * j + qc + 1
                att_box = [None]
                # split long accumulations into <=16-matmul pieces
                k0 = 0
                while k0 < nkq:
                    k1 = min(k0 + 16, nkq)
                    sink(27 * (k1 - k0) + (100 if k1 == nkq else 0),
                         mk_chunk(qc, k0, k1, att_box, k0 == 0, k1 == nkq))
                    k0 = k1

            if h == 2:
                ao_box = [None, None]
                for qc in range(4):
                    sink(450, mk_transpose(j, qc, ao_box))
                for qc in range(4):
                    for ns in range(2):
                        sink(550, mk_proj(j, qc, ns, ao_box))

        def mk_transpose(j, qc, ao_box):
            def clo():
                if qc == 0:
                    ao_box[0] = aoT_pool.tile([128, 512], bf16, tag="ao01",
                                              name="ao01")
                    ao_box[1] = aoT_pool.tile([64, 512], bf16, tag="ao2",
                                              name="ao2")
                tps = next_slot().bitcast(bf16)
                tp1 = tps[:, 0:1024]
                tp2 = tps[:, 1024:2048]
                nc.tensor.transpose(tp1[:, 0:128], attn_t[qc][:, 0:128],
                                    ident[:])
                nc.tensor.transpose(tp2[0:64, 0:128], attn_t[qc][:, 128:192],
                                    ident[:])
                csl = slice(qc * 128, qc * 128 + 128)
                nc.vector.tensor_copy(ao_box[0][:, csl], tp1[:, 0:128])
                nc.vector.tensor_copy(ao_box[1][:, csl], tp2[0:64, 0:128])
            return clo

        def mk_proj(j, qc, ns, ao_box):
            def clo():
                msl = slice(j * 512 + qc * 128, j * 512 + qc * 128 + 128)
                csl = slice(qc * 128, qc * 128 + 128)
                py = next_slot()[:, 0:384]
                nc.tensor.matmul(py, ao_box[0][:, csl],
                                 wp01_sb[:, ns * 384:(ns + 1) * 384],
                                 start=True, stop=False)
                nc.tensor.matmul(py, ao_box[1][:, csl],
                                 wp2_sb[:, ns * 384:(ns + 1) * 384],
                                 start=False, stop=True)
                y_sb = y_pool.tile([128, 384], f32, tag="y", name="ysb")
                nc.vector.tensor_copy(y_sb[:], py)
                nc.sync.dma_start(y_d[msl, ns * 384:(ns + 1) * 384], y_sb[:])
            return clo

        # ------------- main pipeline -------------
        attn_t = [None] * 4  # per-q-chunk staging tiles (rebound per j)
        _, qk0, v0 = emit_qkv(0, xt_pre=xt0)
        for _, fn in qk0[:4]:
            fn()  # bootstrap: q01/k01 chains only
        for item in qk0[4:] + v0:
            push_urgent(*item)  # slot2 + v hide inside stream (0,0)
        parts = {}
        if NT > 1:
            parts[1] = emit_qkv(1)
            for item in parts[1][0]:
                item[1]()  # xt prefetch for j=1 up front
        for j in range(NT):
            for h in range(HPC):
                eb_t = eb_pool.tile([128, (NT * 4) * 512], bf16, tag="eb",
                                    name="eb")
                if h == 0 and j + 1 < NT:
                    for item in parts[j + 1][1]:
                        push_urgent(*item)
                elif h == 1 and j + 1 < NT:
                    for item in parts[j + 1][2]:
                        push_urgent(*item)
                elif h == 2 and j + 2 < NT:
                    parts[j + 2] = emit_qkv(j + 2)
                    for item in parts[j + 2][0]:
                        push_urgent(*item)  # xt prefetch two tiles ahead
                last = (j == NT - 1 and h == 2)
                if last:
                    # chunks 0/1 (and their tails) only need exps through
                    # round nk//2 - 2; emit them mid-stream so only chunks
                    # 2/3 trail the final exp
                    items = []
                    emit_attv(j, h, eb_t,
                              sink=lambda c, f: items.append(f))
                    early = items[0:4] + items[8:10] + items[12:16]
                    rest = [f for f in items if f not in early]
                    nrounds = (4 * j + 4 + 1) // 2
                    emit_stream(j, h, eb_t, tail=lambda: [f() for f in rest],
                                late=(nrounds - 2, early))
                else:
                    emit_stream(j, h, eb_t)
                    emit_attv(j, h, eb_t)
        pop_budget(float("inf"))

    nc.compile()
    return nc


_NC_CACHE = {}


def _get_nc(T):
    if T not in _NC_CACHE:
        _NC_CACHE[T] = build_nc(T)
    return _NC_CACHE[T]


def make_core_inputs(x, W_attn, b_attn, W_proj):
    """Host-side prep: per-core input dicts (see module docstring)."""
    import ml_dtypes
    B, T, _ = x.shape
    xts = [np.ascontiguousarray(x[b].T).astype(ml_dtypes.bfloat16)
           for b in range(B)]
    in_maps = []
    for core in range(N_CORES):
        b = core // (N_CORES // B)
        h0 = HPC * (core % (N_CORES // B))
        c01 = slice(h0 * D, (h0 + 2) * D)      # first two heads
        c2 = slice((h0 + 2) * D, (h0 + 3) * D)  # third head
        # reference splits qkv as (k, q, v): k cols 0:C, q cols C:2C, v 2C:3C
        q01 = W_attn[:, C:2 * C][:, c01]
        k01 = W_attn[:, 0:C][:, c01]
        q2 = W_attn[:, C:2 * C][:, c2]
        k2 = W_attn[:, 0:C][:, c2]
        wv = W_attn[:, 2 * C:3 * C][:, h0 * D:(h0 + 3) * D]
        wqv = np.concatenate([q01, k01, q2, k2, wv], axis=1)
        bqk = np.zeros((128, 3), np.float32)
        bqk[:, 0] = b_attn[C:2 * C][c01]
        bqk[:, 1] = b_attn[0:C][c01]
        bqk[0:64, 2] = b_attn[C:2 * C][c2]
        bqk[64:128, 2] = b_attn[0:C][c2]
        bv = b_attn[2 * C:3 * C][h0 * D:(h0 + 3) * D].reshape(1, 192)
        wp01 = W_proj[h0 * D:(h0 + 2) * D, :]
        wp2 = W_proj[(h0 + 2) * D:(h0 + 3) * D, :]
        in_maps.append({
            "xt": xts[b],
            "wqv": np.ascontiguousarray(wqv).astype(ml_dtypes.bfloat16),
            "bv": np.ascontiguousarray(bv).astype(ml_dtypes.bfloat16),
            "bqk": bqk,
            "wp01": np.ascontiguousarray(wp01).astype(ml_dtypes.bfloat16),
            "wp2": np.ascontiguousarray(wp2).astype(ml_dtypes.bfloat16),
        })
    return in_maps


def kernel(x, W_attn, b_attn, W_proj, b_proj):
    x = np.asarray(x, dtype=np.float32)
    W_attn = np.asarray(W_attn, dtype=np.float32)
    b_attn = np.asarray(b_attn, dtype=np.float32)
    W_proj = np.asarray(W_proj, dtype=np.float32)
    b_proj = np.asarray(b_proj, dtype=np.float32)
    B, T, _ = x.shape

    nc = _get_nc(T)
    in_maps = make_core_inputs(x, W_attn, b_attn, W_proj)
    res = None
    for attempt in range(3):
        try:
            res = run_bass_kernel_spmd(nc, in_maps, list(range(N_CORES)))
            break
        except Exception:
            # transient NRT_EXEC_UNIT_UNRECOVERABLE has been observed once
            # after a prior crashed process; a retry succeeds
            if attempt == 2:
                raise
    global LAST_RUN
    LAST_RUN = res

    gpb = N_CORES // B
    out = np.empty((B, T, C), np.float32)
    for b in range(B):
        acc = res.results[b * gpb]["y"].astype(np.float32)
        for g in range(1, gpb):
            acc = acc + res.results[b * gpb + g]["y"]
        out[b] = acc + b_proj[None, :]
    return out
